# revision 43
# baseline (speedup 1.0000x reference)
"""MultiHeadAttention (B=4, S=2048, D=1024, H=16, causal + key mask) on 8 trn2 cores.

Sharding: Megatron-style tensor parallel over heads. Each core owns 2 heads:
column slices of Wq/Wk/Wv (D x 128), the matching row slice of Wp (128 x D).
Each core computes a partial output y_c = attn_c @ Wp_c; host sums the 8
partials and adds bp.

v4 (engine rebalance around the exp stream):
  - ScalarE is (almost) pure exp: q/k/v projection evictions moved to DVE
    tensor_scalar_add (bias fused, f32 psum -> bf16 sbuf in one op).
  - Normalize path unified for all batches: bf16 reciprocal row -> two bf16
    K=1 broadcast matmuls (213ns each, vs 859ns fp32) into PSUM; the
    normalize muls run on DVE reading the broadcast directly from PSUM.
    Kills the 41.6us of gpsimd DIRECT2D descriptor time and the tail stalls.
  - Output-projection evictions split DVE/ACT (3 of 8 per group on ACT) so
    both stay under the PE roofline.
  - xT is staged in DRAM pre-chunked [128, 16, 8, 512] so each chunk load is
    one contiguous 8KB line per partition (was 8x 1KB lines -> descriptor
    bound at ~11us/chunk).
  - Exp ACT table preloaded with a dummy activation during the proj phase.
  - v3 keeps: bf16 matmuls, row-tiled concurrent score pairs, multiplicative
    0/1 causal mask after exp, one-block score lookahead, PV ones-column
    denominator trick, batch b+1 projections interleaved into attention of
    batch b with lag-2 output projections.
"""

import numpy as np

P = 128
B, S, D, H = 4, 2048, 1024, 16
HD = D // H  # 64
NCORES = 8
HPC = H // NCORES  # 2 heads per core
BS = B * S  # 8192
NB = S // P  # 16 k-blocks per batch
NG = S // 512  # 4 q-groups per batch
NCHUNK = BS // 512  # 16 token chunks

_CACHE = {}


def _build_nc():
    import concourse.mybir as mybir
    from concourse import bacc
    from concourse.tile import TileContext
    from concourse.masks import make_identity
    from contextlib import ExitStack

    f32 = mybir.dt.float32
    bf16 = mybir.dt.bfloat16
    AF = mybir.ActivationFunctionType

    nc = bacc.Bacc("TRN2", target_bir_lowering=False, debug=False,
                   num_devices=NCORES)

    KD = D // P  # 8 contraction chunks
    # pre-chunked x^T: [p, chunk, o, m] = x^T[o*128+p, chunk*512+m]
    xT_d = nc.dram_tensor("xT", [P, NCHUNK, KD, 512], bf16,
                          kind="ExternalInput").ap()
    # weights pre-arranged [p, o, m] = W[o*128+p, m] (contiguous 2KB lines)
    wq_d = nc.dram_tensor("wq", [P, KD, P], bf16, kind="ExternalInput").ap()
    wk_d = nc.dram_tensor("wk", [P, KD, P], bf16, kind="ExternalInput").ap()
    wv_d = nc.dram_tensor("wv", [P, KD, P], bf16, kind="ExternalInput").ap()
    bq_d = nc.dram_tensor("bq", [P, 1], f32, kind="ExternalInput").ap()
    bk_d = nc.dram_tensor("bk", [P, 1], f32, kind="ExternalInput").ap()
    bv_d = nc.dram_tensor("bv", [P, 1], f32, kind="ExternalInput").ap()
    wp_d = nc.dram_tensor("wp", [P, D], bf16, kind="ExternalInput").ap()
    mb_d = nc.dram_tensor("maskb", [P, B * NB], f32, kind="ExternalInput").ap()
    cm_d = nc.dram_tensor("cmask", [P, 4, HPC, 512], bf16,
                          kind="ExternalInput").ap()
    yp_d = nc.dram_tensor("yp", [BS, D], bf16, kind="ExternalOutput").ap()

    with TileContext(nc) as tc:
        with ExitStack() as ctx:
            consts = ctx.enter_context(tc.tile_pool(name="consts", bufs=1))
            big = ctx.enter_context(tc.tile_pool(name="big", bufs=1))
            xpool = ctx.enter_context(tc.tile_pool(name="xpool", bufs=3))
            vtpool = ctx.enter_context(tc.tile_pool(name="vtpool", bufs=2))
            ptpool = ctx.enter_context(tc.tile_pool(name="ptpool", bufs=4))
            pvspool = ctx.enter_context(tc.tile_pool(name="pvs", bufs=2))
            npool = ctx.enter_context(tc.tile_pool(name="npool", bufs=3))
            ypool = ctx.enter_context(tc.tile_pool(name="ypool", bufs=6))
            psum = ctx.enter_context(
                tc.tile_pool(name="psum", bufs=2, space="PSUM"))
            sc2pool = ctx.enter_context(
                tc.tile_pool(name="sc2pool", bufs=2, space="PSUM"))
            pvpool = ctx.enter_context(
                tc.tile_pool(name="pvpool", bufs=2, space="PSUM"))

            # ---- constants (critical path first: wq/bq gate proj chunk 0)
            wq_sb = consts.tile([P, KD, P], bf16, tag="wq")
            wk_sb = consts.tile([P, KD, P], bf16, tag="wk")
            wv_sb = consts.tile([P, KD, P], bf16, tag="wv")
            bq_sb = consts.tile([P, 1], f32, tag="bq")
            bk_sb = consts.tile([P, 1], f32, tag="bk")
            bv_sb = consts.tile([P, 1], f32, tag="bv")
            nc.sync.dma_start(wq_sb[:], wq_d)
            nc.sync.dma_start(bq_sb[:], bq_d)
            nc.sync.dma_start(wk_sb[:], wk_d)
            nc.sync.dma_start(bk_sb[:], bk_d)
            nc.sync.dma_start(wv_sb[:], wv_d)
            nc.sync.dma_start(bv_sb[:], bv_d)
            wp_sb = consts.tile([P, D], bf16, tag="wp")
            nc.sync.dma_start(wp_sb[:], wp_d)
            # rows 64..127 of Wp re-homed at partitions 0..63 for the final
            # group's split output projection (lhsT/rhs base partitions must
            # match)
            wp2_sb = consts.tile([HD, D], bf16, tag="wp2")
            nc.sync.dma_start(wp2_sb[:], wp_d[HD:P, :])
            mb_sb = consts.tile([P, B * NB], f32, tag="mb")
            nc.sync.dma_start(mb_sb[:], mb_d)
            # multiplicative causal masks, [p, j, head, q'] 0/1 bf16
            cm_sb = consts.tile([P, 4, HPC, 512], bf16, tag="cm")
            nc.sync.dma_start(cm_sb[:], cm_d)
            ident = consts.tile([P, P], bf16, tag="ident")
            make_identity(nc, ident[:])
            # ones row on partition 64 (same partition as the PV denominator
            # row) -- bf16 lhsT of the reciprocal-broadcast matmul
            ones64 = consts.tile([P, HD], bf16, tag="ones64")
            nc.vector.memset(ones64[HD:HD + 1, :], 1.0)
            # scratch for the exp ACT-table preload
            warm = consts.tile([P, 1], f32, tag="warm")
            nc.scalar.activation(warm[:], bq_sb[:], AF.Exp)

            # ---- persistent activations (all bf16) ----
            qt_sb = big.tile([P, B, S], bf16, tag="qt")  # Q^T
            kt_sb = big.tile([P, B, S], bf16, tag="kt")  # K^T
            # V in [s, hd] layout + ones col: [p=s%128, b, sblock, h, 65]
            v_sb = big.tile([P, B, NB, HPC, HD + 1], bf16, tag="v")
            at_sb = big.tile([P, B, S], bf16, tag="at")  # attn^T (normalized)
            nc.vector.memset(v_sb[:, :, :, :, HD], 1.0)

            # ---- projections for one 512-row chunk of x ----
            def proj_chunk(c):
                b, sc = divmod(c, NG)
                xt = xpool.tile([P, KD, 512], bf16, tag="xt")
                # gpsimd DMA queue: the sync queue carries the y writes whose
                # in-queue semaphore waits would head-of-line block this load
                nc.gpsimd.dma_start(xt[:], xT_d[:, c, :, :])
                ssl = slice(sc * 512, (sc + 1) * 512)
                for which in range(3):
                    w_sb = (wq_sb, wk_sb, wv_sb)[which]
                    ps = psum.tile([P, 512], f32, tag="ps")
                    for o in range(KD):
                        nc.tensor.matmul(
                            ps[:], lhsT=w_sb[:, o, :], rhs=xt[:, o, :],
                            start=(o == 0), stop=(o == KD - 1))
                    if which == 0:
                        nc.vector.tensor_scalar_add(qt_sb[:, b, ssl], ps[:],
                                                    bq_sb[:])
                    elif which == 1:
                        nc.vector.tensor_scalar_add(kt_sb[:, b, ssl], ps[:],
                                                    bk_sb[:])
                    else:
                        vt = vtpool.tile([P, 512], bf16, tag="vt")
                        nc.vector.tensor_scalar_add(vt[:], ps[:], bv_sb[:])
                        for t in range(4):
                            # shares the "ps" slots (pools size per tag)
                            trp = psum.tile([P, P], bf16, tag="ps")
                            nc.tensor.transpose(
                                trp[:], vt[:, t * P:(t + 1) * P], ident[:])
                            sb_i = sc * 4 + t
                            nc.vector.tensor_copy(
                                v_sb[:, b, sb_i, 0, 0:HD], trp[:, 0:HD])
                            nc.vector.tensor_copy(
                                v_sb[:, b, sb_i, 1, 0:HD],
                                trp[:, HD:2 * HD])

            # ---- output projection for one (b, g) q-group ----
            def outproj(b, g):
                for qc in range(4):
                    q0 = g * 512 + qc * P
                    r0 = b * S + q0
                    y_sb = ypool.tile([P, 2, 512], bf16, tag="y",
                                      name=f"y_{b}_{g}_{qc}")
                    for half in range(2):
                        yp_ps = psum.tile([P, 512], f32, tag="ps",
                                          name=f"yps_{b}_{g}_{qc}_{half}")
                        nc.tensor.matmul(
                            yp_ps[:],
                            lhsT=at_sb[:, b, q0:q0 + P],
                            rhs=wp_sb[:, half * 512:(half + 1) * 512],
                            start=True, stop=True)
                        # all evictions on DVE: ScalarE is strict FIFO, so
                        # an eviction waiting on its outproj matmul would
                        # head-of-line block the next group's exp
                        nc.vector.tensor_copy(y_sb[:, half, :], yp_ps[:])
                    nc.sync.dma_start(
                        yp_d[r0:r0 + P, :],
                        y_sb[:].rearrange("p a n -> p (a n)"))

            # ---- attention for one (b, g) q-group ----
            def attn_group(b, g, pending, split_out=False, tail=False):
                gsl = slice(g * 512, (g + 1) * 512)
                nkb = 4 * (g + 1)
                order = list(range(nkb))
                pvs = [pvpool.tile([P, 512], f32, tag="pv",
                                   name=f"pv_{b}_{g}_{h}")
                       for h in range(HPC)]

                def scores(kb):
                    j = kb - 4 * g
                    # diagonal blocks: q < 128*j is fully masked
                    qo = 128 * max(j, 0)
                    sc2 = sc2pool.tile([P, HPC, 512], f32, tag="sc2",
                                       name=f"sc2_{b}_{g}_{kb}")
                    for h in range(HPC):
                        hsl = slice(h * HD, (h + 1) * HD)
                        nc.tensor.matmul(
                            sc2[:, h, qo:512],
                            lhsT=kt_sb[hsl, b, kb * P:(kb + 1) * P],
                            rhs=qt_sb[hsl, b, g * 512 + qo:(g + 1) * 512],
                            start=True, stop=True)
                    return sc2, kb, j, qo

                # hoist this group's first two score-pairs past the
                # boundary proj/outproj bursts (priority = emission order,
                # offset 60 ~ one proj chunk + drain) so the exp stream
                # restarts immediately at group boundaries
                with tc.high_priority(offset=150):
                    cur = scores(order[0])
                for i, kb in enumerate(order):
                    if i + 1 < nkb:
                        if i == 0:
                            with tc.high_priority(offset=150):
                                nxt = scores(order[1])
                        else:
                            nxt = scores(order[i + 1])
                    else:
                        nxt = None
                    sc2, _, j, qo = cur
                    col = b * NB + kb
                    pt = ptpool.tile([P, HPC, 512], bf16, tag="pt")
                    if qo == 0:
                        nc.scalar.activation(pt[:], sc2[:], AF.Exp,
                                             bias=mb_sb[:, col:col + 1])
                    else:
                        nc.scalar.activation(pt[:, :, qo:512],
                                             sc2[:, :, qo:512], AF.Exp,
                                             bias=mb_sb[:, col:col + 1])
                    if j >= 0:
                        # causal mask: only the 128-col strip [qo, qo+128)
                        # is triangular -- mask just that strip so the wide
                        # clean part of PV never waits on the mask-mul
                        nc.vector.tensor_mul(pt[:, :, qo:qo + P],
                                             pt[:, :, qo:qo + P],
                                             cm_sb[:, j, :, qo:qo + P])
                    for h in range(HPC):
                        if j >= 0 and qo + P < 512:
                            # clean columns: chain is exp -> PV directly
                            nc.tensor.matmul(
                                pvs[h][0:HD + 1, qo + P:512],
                                lhsT=v_sb[:, b, kb, h, :],
                                rhs=pt[:, h, qo + P:512],
                                start=(i == 0), stop=False)
                            nc.tensor.matmul(
                                pvs[h][0:HD + 1, qo:qo + P],
                                lhsT=v_sb[:, b, kb, h, :],
                                rhs=pt[:, h, qo:qo + P],
                                start=False, stop=(i == nkb - 1))
                        else:
                            nc.tensor.matmul(
                                pvs[h][0:HD + 1, qo:512],
                                lhsT=v_sb[:, b, kb, h, :],
                                rhs=pt[:, h, qo:512],
                                start=(i == 0), stop=(i == nkb - 1))
                    cur = nxt
                # evict PV psums immediately so the banks recycle without
                # waiting on the normalize chain
                pvs_sb = pvspool.tile([P, HPC, 512], f32, tag="pvs")
                for h in range(HPC):
                    nc.vector.tensor_copy(pvs_sb[0:HD + 1, h, :],
                                          pvs[h][0:HD + 1, :])
                if not split_out:
                    pending.append((b, g))
                # ---- normalize: 1/denom (row 64, bf16) -> K=1 bf16 matmul
                # broadcast into PSUM -> DVE muls reading PSUM ----
                # reciprocal_approx_fast misbehaves on single-partition
                # slices -- run it over the full tile (unused rows discarded)
                rcp = npool.tile([P, HPC, 512], f32, tag="rcp")
                nc.vector.reciprocal_approx_fast(rcp[:], pvs_sb[:])
                if not tail:
                    # slack-tolerant path (outproj lag hides the 6-10us DMA
                    # latency): partition-broadcast via DMA descriptors and
                    # muls on the otherwise-idle GpSimd -- zero PE cost and
                    # near-zero DVE cost
                    tmp = npool.tile([HD, 512], bf16, tag="tmp")
                    dbc = npool.tile([HD, HPC, 512], f32, tag="dbc")
                    for h in range(HPC):
                        nc.gpsimd.dma_start(
                            dbc[:, h, :],
                            rcp[HD:HD + 1, h, None, :]
                            .to_broadcast((1, HD, 512)))
                    nc.gpsimd.tensor_mul(at_sb[0:HD, b, gsl],
                                         pvs_sb[0:HD, 0, :], dbc[:, 0, :])
                    nc.gpsimd.tensor_mul(tmp[:], pvs_sb[0:HD, 1, :],
                                         dbc[:, 1, :])
                    nc.gpsimd.dma_start(at_sb[HD:2 * HD, b, gsl], tmp[:])
                    return None
                # tail groups: short matmul-broadcast chain
                # bf16 copy of the denominator-reciprocal row: keeps the
                # broadcast matmul at 1 cyc/row (fp32 rhs would be 4x)
                rcpb = npool.tile([P, HPC, 512], bf16, tag="rcpb")
                nc.vector.tensor_copy(rcpb[HD:HD + 1, :, :],
                                      rcp[HD:HD + 1, :, :])
                bc = [pvpool.tile([P, 512], f32, tag="pv",
                                  name=f"bc_{b}_{g}_{h}")
                      for h in range(HPC)]
                for h in range(HPC):
                    nc.tensor.matmul(
                        bc[h][0:HD, :], lhsT=ones64[HD:HD + 1, :],
                        rhs=rcpb[HD:HD + 1, h, :], start=True, stop=True)
                tmp = npool.tile([HD, 512], bf16, tag="tmp")
                if split_out:
                    # final group: keep both halves at partitions 0..63 and
                    # feed the split output projection directly -- no at_sb
                    # partition-shift DMA in the tail chain
                    a0 = npool.tile([HD, 512], bf16, tag="a0")
                    nc.vector.tensor_mul(a0[:], pvs_sb[0:HD, 0, :],
                                         bc[0][0:HD, :])
                    nc.vector.tensor_mul(tmp[:], pvs_sb[0:HD, 1, :],
                                         bc[1][0:HD, :])
                    return a0, tmp
                nc.vector.tensor_mul(at_sb[0:HD, b, gsl],
                                     pvs_sb[0:HD, 0, :], bc[0][0:HD, :])
                nc.vector.tensor_mul(tmp[:], pvs_sb[0:HD, 1, :],
                                     bc[1][0:HD, :])
                nc.gpsimd.dma_start(at_sb[HD:2 * HD, b, gsl], tmp[:])
                return None

            # ---- split output projection for the final group: two k=64
            # accumulating matmuls per psum, lhsT halves at partitions 0-63
            def outproj_split(b, g, a0, a1):
                for qc in range(4):
                    q0 = g * 512 + qc * P
                    r0 = b * S + q0
                    qsl = slice(qc * P, (qc + 1) * P)
                    y_sb = ypool.tile([P, 2, 512], bf16, tag="y",
                                      name=f"ys_{b}_{g}_{qc}")
                    for half in range(2):
                        nsl = slice(half * 512, (half + 1) * 512)
                        yp_ps = psum.tile([P, 512], f32, tag="ps",
                                          name=f"yss_{b}_{g}_{qc}_{half}")
                        nc.tensor.matmul(
                            yp_ps[:], lhsT=a0[:, qsl],
                            rhs=wp_sb[0:HD, nsl], start=True, stop=False)
                        nc.tensor.matmul(
                            yp_ps[:], lhsT=a1[:, qsl],
                            rhs=wp2_sb[:, nsl], start=False, stop=True)
                        nc.vector.tensor_copy(y_sb[:, half, :], yp_ps[:])
                    nc.sync.dma_start(
                        yp_d[r0:r0 + P, :],
                        y_sb[:].rearrange("p a n -> p (a n)"))

            # ---- schedule: uniform pipeline. proj chunks run 2 groups ahead
            # of the attention that consumes them, so batch-0 attention (and
            # its exp stream) starts ~8us in instead of after a dead 22us
            # proj-only phase. Output projections are deferred into the
            # ACT-bound batch-3 window via the drain table. ----
            DRAIN = [[0, 0, 1, 1], [0, 1, 1, 1], [0, 0, 1, 1], [2, 2, 2, 9]]
            proj_chunk(0)
            pending = []
            for b in range(B):
                for g in range(NG):
                    last = (b == B - 1 and g == NG - 1)
                    ret = attn_group(b, g, pending, split_out=last,
                                     tail=(b == B - 1 and g >= NG - 2))
                    for _ in range(DRAIN[b][g]):
                        if pending:
                            outproj(*pending.pop(0))
                    if last:
                        outproj_split(b, g, *ret)
                    # 1-ahead proj cadence (keeps late chunks as batch-3 PE
                    # filler), except batch-crossing chunks go 2 ahead: the
                    # next batch's first scores depend on its first chunk,
                    # and a 1-ahead emission stalls the exp stream 10-19us
                    # at every batch boundary
                    gi = NG * b + g
                    cs = [gi + 1, gi + 2] if g == NG - 2 else (
                        [] if g == NG - 1 else [gi + 1])
                    for c in cs:
                        if c < NCHUNK:
                            proj_chunk(c)

    nc.compile()
    return nc


def _get_nc():
    if "nc" not in _CACHE:
        _CACHE["nc"] = _build_nc()
    return _CACHE["nc"]


def make_in_maps(x, attention_mask, Wq, bq, Wk, bk, Wv, bv, Wp, bp):
    """Host-side sharding: build the 8 per-core device input maps."""
    import ml_dtypes
    bf16 = ml_dtypes.bfloat16
    KD8 = D // P
    x = np.asarray(x, dtype=np.float32)
    scale = np.float32(1.0 / np.sqrt(HD))
    xT = x.reshape(BS, D).T.astype(bf16)  # [D, BS]
    # pre-chunked layout: [p, chunk, o, m] = xT[o*128+p, chunk*512+m]
    xTc = np.ascontiguousarray(
        xT.reshape(D // P, P, NCHUNK, 512).transpose(1, 2, 0, 3))
    mb = (np.asarray(attention_mask).astype(np.float32) - 1.0) * np.float32(1e9)
    mb = np.ascontiguousarray(
        mb.reshape(B, NB, P).transpose(2, 0, 1).reshape(P, B * NB))
    # multiplicative causal masks: 1 where 128*j + p <= q', else 0;
    # duplicated for the two heads: [128, 4, 2, 512]
    pp = np.arange(P)[:, None]
    ff = np.arange(512)[None, :]
    cm = np.stack(
        [np.where(P * j + pp <= ff, 1.0, 0.0).astype(bf16)
         for j in range(4)], axis=1)  # [128, 4, 512]
    cm = np.ascontiguousarray(
        np.broadcast_to(cm[:, :, None, :], (P, 4, HPC, 512)))

    Wq = (np.asarray(Wq, np.float32) * scale).astype(bf16)
    bq = np.asarray(bq, np.float32) * scale
    Wk = np.asarray(Wk, np.float32).astype(bf16)
    bk = np.asarray(bk, np.float32)
    Wv = np.asarray(Wv, np.float32).astype(bf16)
    bv = np.asarray(bv, np.float32)
    Wp = np.asarray(Wp, np.float32).astype(bf16)

    def wrearr(w, cs):
        # [1024, 128] core slice -> [p, o, m] = W[o*128+p, m]
        return np.ascontiguousarray(
            w[:, cs].reshape(KD8, P, P).transpose(1, 0, 2))

    in_maps = []
    for c in range(NCORES):
        cs = slice(c * P, (c + 1) * P)
        in_maps.append({
            "xT": xTc,
            "wq": wrearr(Wq, cs),
            "wk": wrearr(Wk, cs),
            "wv": wrearr(Wv, cs),
            "bq": np.ascontiguousarray(bq[cs].reshape(P, 1)),
            "bk": np.ascontiguousarray(bk[cs].reshape(P, 1)),
            "bv": np.ascontiguousarray(bv[cs].reshape(P, 1)),
            "wp": np.ascontiguousarray(Wp[cs, :]),
            "maskb": mb,
            "cmask": cm,
        })
    return in_maps


def run(inputs, trace=False, tmpdir=None):
    """Compile (cached) + run on 8 cores. Returns (output, BassKernelResults)."""
    from concourse import bass_utils
    nc = _get_nc()
    in_maps = make_in_maps(**inputs)
    kwargs = {}
    if trace:
        kwargs = dict(trace=True, tmpdir=tmpdir)
    res = bass_utils.run_bass_kernel_spmd(
        nc, in_maps, core_ids=list(range(NCORES)), **kwargs)
    acc = np.zeros((BS, D), dtype=np.float32)
    for r in res.results:
        acc += r["yp"].astype(np.float32)
    out = acc + np.asarray(inputs["bp"], np.float32)[None, :]
    return out.reshape(B, S, D), res


def kernel(**inputs) -> np.ndarray:
    out, _ = run(inputs, trace=False)
    return out


# revision 44
# speedup vs baseline: 1.0002x; 1.0002x over previous
"""MultiHeadAttention (B=4, S=2048, D=1024, H=16, causal + key mask) on 8 trn2 cores.

Sharding: Megatron-style tensor parallel over heads. Each core owns 2 heads:
column slices of Wq/Wk/Wv (D x 128), the matching row slice of Wp (128 x D).
Each core computes a partial output y_c = attn_c @ Wp_c; host sums the 8
partials and adds bp.

v4 (engine rebalance around the exp stream):
  - ScalarE is (almost) pure exp: q/k/v projection evictions moved to DVE
    tensor_scalar_add (bias fused, f32 psum -> bf16 sbuf in one op).
  - Normalize path unified for all batches: bf16 reciprocal row -> two bf16
    K=1 broadcast matmuls (213ns each, vs 859ns fp32) into PSUM; the
    normalize muls run on DVE reading the broadcast directly from PSUM.
    Kills the 41.6us of gpsimd DIRECT2D descriptor time and the tail stalls.
  - Output-projection evictions split DVE/ACT (3 of 8 per group on ACT) so
    both stay under the PE roofline.
  - xT is staged in DRAM pre-chunked [128, 16, 8, 512] so each chunk load is
    one contiguous 8KB line per partition (was 8x 1KB lines -> descriptor
    bound at ~11us/chunk).
  - Exp ACT table preloaded with a dummy activation during the proj phase.
  - v3 keeps: bf16 matmuls, row-tiled concurrent score pairs, multiplicative
    0/1 causal mask after exp, one-block score lookahead, PV ones-column
    denominator trick, batch b+1 projections interleaved into attention of
    batch b with lag-2 output projections.
"""

import numpy as np

P = 128
B, S, D, H = 4, 2048, 1024, 16
HD = D // H  # 64
NCORES = 8
HPC = H // NCORES  # 2 heads per core
BS = B * S  # 8192
NB = S // P  # 16 k-blocks per batch
NG = S // 512  # 4 q-groups per batch
NCHUNK = BS // 512  # 16 token chunks

_CACHE = {}


def _build_nc():
    import concourse.mybir as mybir
    from concourse import bacc
    from concourse.tile import TileContext
    from concourse.masks import make_identity
    from contextlib import ExitStack

    f32 = mybir.dt.float32
    bf16 = mybir.dt.bfloat16
    AF = mybir.ActivationFunctionType

    nc = bacc.Bacc("TRN2", target_bir_lowering=False, debug=False,
                   num_devices=NCORES)

    KD = D // P  # 8 contraction chunks
    # pre-chunked x^T: [p, chunk, o, m] = x^T[o*128+p, chunk*512+m]
    xT_d = nc.dram_tensor("xT", [P, NCHUNK, KD, 512], bf16,
                          kind="ExternalInput").ap()
    # weights pre-arranged [p, o, m] = W[o*128+p, m] (contiguous 2KB lines)
    wq_d = nc.dram_tensor("wq", [P, KD, P], bf16, kind="ExternalInput").ap()
    wk_d = nc.dram_tensor("wk", [P, KD, P], bf16, kind="ExternalInput").ap()
    wv_d = nc.dram_tensor("wv", [P, KD, P], bf16, kind="ExternalInput").ap()
    bq_d = nc.dram_tensor("bq", [P, 1], f32, kind="ExternalInput").ap()
    bk_d = nc.dram_tensor("bk", [P, 1], f32, kind="ExternalInput").ap()
    bv_d = nc.dram_tensor("bv", [P, 1], f32, kind="ExternalInput").ap()
    wp_d = nc.dram_tensor("wp", [P, D], bf16, kind="ExternalInput").ap()
    mb_d = nc.dram_tensor("maskb", [P, B * NB], f32, kind="ExternalInput").ap()
    cm_d = nc.dram_tensor("cmask", [P, 4, HPC, 512], bf16,
                          kind="ExternalInput").ap()
    yp_d = nc.dram_tensor("yp", [BS, D], bf16, kind="ExternalOutput").ap()

    with TileContext(nc) as tc:
        with ExitStack() as ctx:
            consts = ctx.enter_context(tc.tile_pool(name="consts", bufs=1))
            big = ctx.enter_context(tc.tile_pool(name="big", bufs=1))
            xpool = ctx.enter_context(tc.tile_pool(name="xpool", bufs=3))
            vtpool = ctx.enter_context(tc.tile_pool(name="vtpool", bufs=2))
            ptpool = ctx.enter_context(tc.tile_pool(name="ptpool", bufs=4))
            pvspool = ctx.enter_context(tc.tile_pool(name="pvs", bufs=2))
            npool = ctx.enter_context(tc.tile_pool(name="npool", bufs=3))
            ypool = ctx.enter_context(tc.tile_pool(name="ypool", bufs=6))
            psum = ctx.enter_context(
                tc.tile_pool(name="psum", bufs=2, space="PSUM"))
            sc2pool = ctx.enter_context(
                tc.tile_pool(name="sc2pool", bufs=2, space="PSUM"))
            pvpool = ctx.enter_context(
                tc.tile_pool(name="pvpool", bufs=2, space="PSUM"))

            # ---- constants (critical path first: wq/bq gate proj chunk 0)
            wq_sb = consts.tile([P, KD, P], bf16, tag="wq")
            wk_sb = consts.tile([P, KD, P], bf16, tag="wk")
            wv_sb = consts.tile([P, KD, P], bf16, tag="wv")
            bq_sb = consts.tile([P, 1], f32, tag="bq")
            bk_sb = consts.tile([P, 1], f32, tag="bk")
            bv_sb = consts.tile([P, 1], f32, tag="bv")
            nc.sync.dma_start(wq_sb[:], wq_d)
            nc.sync.dma_start(bq_sb[:], bq_d)
            nc.sync.dma_start(wk_sb[:], wk_d)
            nc.sync.dma_start(bk_sb[:], bk_d)
            nc.sync.dma_start(wv_sb[:], wv_d)
            nc.sync.dma_start(bv_sb[:], bv_d)
            wp_sb = consts.tile([P, D], bf16, tag="wp")
            nc.sync.dma_start(wp_sb[:], wp_d)
            # rows 64..127 of Wp re-homed at partitions 0..63 for the final
            # group's split output projection (lhsT/rhs base partitions must
            # match)
            wp2_sb = consts.tile([HD, D], bf16, tag="wp2")
            nc.sync.dma_start(wp2_sb[:], wp_d[HD:P, :])
            mb_sb = consts.tile([P, B * NB], f32, tag="mb")
            nc.sync.dma_start(mb_sb[:], mb_d)
            # multiplicative causal masks, [p, j, head, q'] 0/1 bf16
            cm_sb = consts.tile([P, 4, HPC, 512], bf16, tag="cm")
            nc.sync.dma_start(cm_sb[:], cm_d)
            ident = consts.tile([P, P], bf16, tag="ident")
            make_identity(nc, ident[:])
            # ones row on partition 64 (same partition as the PV denominator
            # row) -- bf16 lhsT of the reciprocal-broadcast matmul
            ones64 = consts.tile([P, HD], bf16, tag="ones64")
            nc.vector.memset(ones64[HD:HD + 1, :], 1.0)
            # scratch for the exp ACT-table preload
            warm = consts.tile([P, 1], f32, tag="warm")
            nc.scalar.activation(warm[:], bq_sb[:], AF.Exp)

            # ---- persistent activations (all bf16) ----
            qt_sb = big.tile([P, B, S], bf16, tag="qt")  # Q^T
            kt_sb = big.tile([P, B, S], bf16, tag="kt")  # K^T
            # V in [s, hd] layout + ones col: [p=s%128, b, sblock, h, 65]
            v_sb = big.tile([P, B, NB, HPC, HD + 1], bf16, tag="v")
            at_sb = big.tile([P, B, S], bf16, tag="at")  # attn^T (normalized)
            nc.vector.memset(v_sb[:, :, :, :, HD], 1.0)

            # ---- projections for one 512-row chunk of x ----
            def proj_chunk(c):
                b, sc = divmod(c, NG)
                xt = xpool.tile([P, KD, 512], bf16, tag="xt")
                # gpsimd DMA queue: the sync queue carries the y writes whose
                # in-queue semaphore waits would head-of-line block this load
                nc.gpsimd.dma_start(xt[:], xT_d[:, c, :, :])
                ssl = slice(sc * 512, (sc + 1) * 512)
                for which in range(3):
                    w_sb = (wq_sb, wk_sb, wv_sb)[which]
                    ps = psum.tile([P, 512], f32, tag="ps")
                    for o in range(KD):
                        nc.tensor.matmul(
                            ps[:], lhsT=w_sb[:, o, :], rhs=xt[:, o, :],
                            start=(o == 0), stop=(o == KD - 1))
                    if which == 0:
                        nc.vector.tensor_scalar_add(qt_sb[:, b, ssl], ps[:],
                                                    bq_sb[:])
                    elif which == 1:
                        nc.vector.tensor_scalar_add(kt_sb[:, b, ssl], ps[:],
                                                    bk_sb[:])
                    else:
                        vt = vtpool.tile([P, 512], bf16, tag="vt")
                        nc.vector.tensor_scalar_add(vt[:], ps[:], bv_sb[:])
                        for t in range(4):
                            # shares the "ps" slots (pools size per tag)
                            trp = psum.tile([P, P], bf16, tag="ps")
                            nc.tensor.transpose(
                                trp[:], vt[:, t * P:(t + 1) * P], ident[:])
                            sb_i = sc * 4 + t
                            nc.vector.tensor_copy(
                                v_sb[:, b, sb_i, 0, 0:HD], trp[:, 0:HD])
                            nc.vector.tensor_copy(
                                v_sb[:, b, sb_i, 1, 0:HD],
                                trp[:, HD:2 * HD])

            # ---- output projection for one (b, g) q-group ----
            def outproj(b, g):
                for qc in range(4):
                    q0 = g * 512 + qc * P
                    r0 = b * S + q0
                    y_sb = ypool.tile([P, 2, 512], bf16, tag="y",
                                      name=f"y_{b}_{g}_{qc}")
                    for half in range(2):
                        yp_ps = psum.tile([P, 512], f32, tag="ps",
                                          name=f"yps_{b}_{g}_{qc}_{half}")
                        nc.tensor.matmul(
                            yp_ps[:],
                            lhsT=at_sb[:, b, q0:q0 + P],
                            rhs=wp_sb[:, half * 512:(half + 1) * 512],
                            start=True, stop=True)
                        # all evictions on DVE: ScalarE is strict FIFO, so
                        # an eviction waiting on its outproj matmul would
                        # head-of-line block the next group's exp
                        nc.vector.tensor_copy(y_sb[:, half, :], yp_ps[:])
                    nc.sync.dma_start(
                        yp_d[r0:r0 + P, :],
                        y_sb[:].rearrange("p a n -> p (a n)"))

            # ---- attention for one (b, g) q-group ----
            def attn_group(b, g, pending, split_out=False, tail=False):
                gsl = slice(g * 512, (g + 1) * 512)
                nkb = 4 * (g + 1)
                order = list(range(nkb))
                pvs = [pvpool.tile([P, 512], f32, tag="pv",
                                   name=f"pv_{b}_{g}_{h}")
                       for h in range(HPC)]

                def scores(kb):
                    j = kb - 4 * g
                    # diagonal blocks: q < 128*j is fully masked
                    qo = 128 * max(j, 0)
                    sc2 = sc2pool.tile([P, HPC, 512], f32, tag="sc2",
                                       name=f"sc2_{b}_{g}_{kb}")
                    for h in range(HPC):
                        hsl = slice(h * HD, (h + 1) * HD)
                        nc.tensor.matmul(
                            sc2[:, h, qo:512],
                            lhsT=kt_sb[hsl, b, kb * P:(kb + 1) * P],
                            rhs=qt_sb[hsl, b, g * 512 + qo:(g + 1) * 512],
                            start=True, stop=True)
                    return sc2, kb, j, qo

                # hoist this group's first two score-pairs past the
                # boundary proj/outproj bursts (priority = emission order,
                # offset 60 ~ one proj chunk + drain) so the exp stream
                # restarts immediately at group boundaries
                with tc.high_priority(offset=60):
                    cur = scores(order[0])
                for i, kb in enumerate(order):
                    if i + 1 < nkb:
                        if i == 0:
                            with tc.high_priority(offset=60):
                                nxt = scores(order[1])
                        else:
                            nxt = scores(order[i + 1])
                    else:
                        nxt = None
                    sc2, _, j, qo = cur
                    col = b * NB + kb
                    pt = ptpool.tile([P, HPC, 512], bf16, tag="pt")
                    if qo == 0:
                        nc.scalar.activation(pt[:], sc2[:], AF.Exp,
                                             bias=mb_sb[:, col:col + 1])
                    else:
                        nc.scalar.activation(pt[:, :, qo:512],
                                             sc2[:, :, qo:512], AF.Exp,
                                             bias=mb_sb[:, col:col + 1])
                    if j >= 0:
                        # causal mask: only the 128-col strip [qo, qo+128)
                        # is triangular -- mask just that strip so the wide
                        # clean part of PV never waits on the mask-mul
                        nc.vector.tensor_mul(pt[:, :, qo:qo + P],
                                             pt[:, :, qo:qo + P],
                                             cm_sb[:, j, :, qo:qo + P])
                    for h in range(HPC):
                        if j >= 0 and qo + P < 512:
                            # clean columns: chain is exp -> PV directly
                            nc.tensor.matmul(
                                pvs[h][0:HD + 1, qo + P:512],
                                lhsT=v_sb[:, b, kb, h, :],
                                rhs=pt[:, h, qo + P:512],
                                start=(i == 0), stop=False)
                            nc.tensor.matmul(
                                pvs[h][0:HD + 1, qo:qo + P],
                                lhsT=v_sb[:, b, kb, h, :],
                                rhs=pt[:, h, qo:qo + P],
                                start=False, stop=(i == nkb - 1))
                        else:
                            nc.tensor.matmul(
                                pvs[h][0:HD + 1, qo:512],
                                lhsT=v_sb[:, b, kb, h, :],
                                rhs=pt[:, h, qo:512],
                                start=(i == 0), stop=(i == nkb - 1))
                    cur = nxt
                # evict PV psums immediately so the banks recycle without
                # waiting on the normalize chain
                pvs_sb = pvspool.tile([P, HPC, 512], f32, tag="pvs")
                for h in range(HPC):
                    nc.vector.tensor_copy(pvs_sb[0:HD + 1, h, :],
                                          pvs[h][0:HD + 1, :])
                if not split_out:
                    pending.append((b, g))
                # ---- normalize: 1/denom (row 64, bf16) -> K=1 bf16 matmul
                # broadcast into PSUM -> DVE muls reading PSUM ----
                # reciprocal_approx_fast misbehaves on single-partition
                # slices -- run it over the full tile (unused rows discarded)
                rcp = npool.tile([P, HPC, 512], f32, tag="rcp")
                nc.vector.reciprocal_approx_fast(rcp[:], pvs_sb[:])
                if not tail:
                    # slack-tolerant path (outproj lag hides the 6-10us DMA
                    # latency): partition-broadcast via DMA descriptors and
                    # muls on the otherwise-idle GpSimd -- zero PE cost and
                    # near-zero DVE cost
                    tmp = npool.tile([HD, 512], bf16, tag="tmp")
                    dbc = npool.tile([HD, HPC, 512], f32, tag="dbc")
                    for h in range(HPC):
                        nc.gpsimd.dma_start(
                            dbc[:, h, :],
                            rcp[HD:HD + 1, h, None, :]
                            .to_broadcast((1, HD, 512)))
                    nc.gpsimd.tensor_mul(at_sb[0:HD, b, gsl],
                                         pvs_sb[0:HD, 0, :], dbc[:, 0, :])
                    nc.gpsimd.tensor_mul(tmp[:], pvs_sb[0:HD, 1, :],
                                         dbc[:, 1, :])
                    nc.gpsimd.dma_start(at_sb[HD:2 * HD, b, gsl], tmp[:])
                    return None
                # tail groups: short matmul-broadcast chain
                # bf16 copy of the denominator-reciprocal row: keeps the
                # broadcast matmul at 1 cyc/row (fp32 rhs would be 4x)
                rcpb = npool.tile([P, HPC, 512], bf16, tag="rcpb")
                nc.vector.tensor_copy(rcpb[HD:HD + 1, :, :],
                                      rcp[HD:HD + 1, :, :])
                bc = [pvpool.tile([P, 512], f32, tag="pv",
                                  name=f"bc_{b}_{g}_{h}")
                      for h in range(HPC)]
                for h in range(HPC):
                    nc.tensor.matmul(
                        bc[h][0:HD, :], lhsT=ones64[HD:HD + 1, :],
                        rhs=rcpb[HD:HD + 1, h, :], start=True, stop=True)
                tmp = npool.tile([HD, 512], bf16, tag="tmp")
                if split_out:
                    # final group: keep both halves at partitions 0..63 and
                    # feed the split output projection directly -- no at_sb
                    # partition-shift DMA in the tail chain
                    a0 = npool.tile([HD, 512], bf16, tag="a0")
                    nc.vector.tensor_mul(a0[:], pvs_sb[0:HD, 0, :],
                                         bc[0][0:HD, :])
                    nc.vector.tensor_mul(tmp[:], pvs_sb[0:HD, 1, :],
                                         bc[1][0:HD, :])
                    return a0, tmp
                nc.vector.tensor_mul(at_sb[0:HD, b, gsl],
                                     pvs_sb[0:HD, 0, :], bc[0][0:HD, :])
                nc.vector.tensor_mul(tmp[:], pvs_sb[0:HD, 1, :],
                                     bc[1][0:HD, :])
                nc.gpsimd.dma_start(at_sb[HD:2 * HD, b, gsl], tmp[:])
                return None

            # ---- split output projection for the final group: two k=64
            # accumulating matmuls per psum, lhsT halves at partitions 0-63
            def outproj_split(b, g, a0, a1):
                for qc in range(4):
                    q0 = g * 512 + qc * P
                    r0 = b * S + q0
                    qsl = slice(qc * P, (qc + 1) * P)
                    y_sb = ypool.tile([P, 2, 512], bf16, tag="y",
                                      name=f"ys_{b}_{g}_{qc}")
                    for half in range(2):
                        nsl = slice(half * 512, (half + 1) * 512)
                        yp_ps = psum.tile([P, 512], f32, tag="ps",
                                          name=f"yss_{b}_{g}_{qc}_{half}")
                        nc.tensor.matmul(
                            yp_ps[:], lhsT=a0[:, qsl],
                            rhs=wp_sb[0:HD, nsl], start=True, stop=False)
                        nc.tensor.matmul(
                            yp_ps[:], lhsT=a1[:, qsl],
                            rhs=wp2_sb[:, nsl], start=False, stop=True)
                        nc.vector.tensor_copy(y_sb[:, half, :], yp_ps[:])
                    nc.sync.dma_start(
                        yp_d[r0:r0 + P, :],
                        y_sb[:].rearrange("p a n -> p (a n)"))

            # ---- schedule: uniform pipeline. proj chunks run 2 groups ahead
            # of the attention that consumes them, so batch-0 attention (and
            # its exp stream) starts ~8us in instead of after a dead 22us
            # proj-only phase. Output projections are deferred into the
            # ACT-bound batch-3 window via the drain table. ----
            DRAIN = [[0, 0, 1, 1], [0, 1, 1, 1], [0, 0, 1, 1], [2, 2, 2, 9]]
            proj_chunk(0)
            pending = []
            for b in range(B):
                for g in range(NG):
                    last = (b == B - 1 and g == NG - 1)
                    ret = attn_group(b, g, pending, split_out=last,
                                     tail=(b == B - 1 and g >= NG - 2))
                    for _ in range(DRAIN[b][g]):
                        if pending:
                            outproj(*pending.pop(0))
                    if last:
                        outproj_split(b, g, *ret)
                    # 1-ahead proj cadence (keeps late chunks as batch-3 PE
                    # filler), except batch-crossing chunks go 2 ahead: the
                    # next batch's first scores depend on its first chunk,
                    # and a 1-ahead emission stalls the exp stream 10-19us
                    # at every batch boundary
                    gi = NG * b + g
                    cs = [gi + 1, gi + 2] if g == NG - 2 else (
                        [] if g == NG - 1 else [gi + 1])
                    for c in cs:
                        if c < NCHUNK:
                            proj_chunk(c)

    nc.compile()
    return nc


def _get_nc():
    if "nc" not in _CACHE:
        _CACHE["nc"] = _build_nc()
    return _CACHE["nc"]


def make_in_maps(x, attention_mask, Wq, bq, Wk, bk, Wv, bv, Wp, bp):
    """Host-side sharding: build the 8 per-core device input maps."""
    import ml_dtypes
    bf16 = ml_dtypes.bfloat16
    KD8 = D // P
    x = np.asarray(x, dtype=np.float32)
    scale = np.float32(1.0 / np.sqrt(HD))
    xT = x.reshape(BS, D).T.astype(bf16)  # [D, BS]
    # pre-chunked layout: [p, chunk, o, m] = xT[o*128+p, chunk*512+m]
    xTc = np.ascontiguousarray(
        xT.reshape(D // P, P, NCHUNK, 512).transpose(1, 2, 0, 3))
    mb = (np.asarray(attention_mask).astype(np.float32) - 1.0) * np.float32(1e9)
    mb = np.ascontiguousarray(
        mb.reshape(B, NB, P).transpose(2, 0, 1).reshape(P, B * NB))
    # multiplicative causal masks: 1 where 128*j + p <= q', else 0;
    # duplicated for the two heads: [128, 4, 2, 512]
    pp = np.arange(P)[:, None]
    ff = np.arange(512)[None, :]
    cm = np.stack(
        [np.where(P * j + pp <= ff, 1.0, 0.0).astype(bf16)
         for j in range(4)], axis=1)  # [128, 4, 512]
    cm = np.ascontiguousarray(
        np.broadcast_to(cm[:, :, None, :], (P, 4, HPC, 512)))

    Wq = (np.asarray(Wq, np.float32) * scale).astype(bf16)
    bq = np.asarray(bq, np.float32) * scale
    Wk = np.asarray(Wk, np.float32).astype(bf16)
    bk = np.asarray(bk, np.float32)
    Wv = np.asarray(Wv, np.float32).astype(bf16)
    bv = np.asarray(bv, np.float32)
    Wp = np.asarray(Wp, np.float32).astype(bf16)

    def wrearr(w, cs):
        # [1024, 128] core slice -> [p, o, m] = W[o*128+p, m]
        return np.ascontiguousarray(
            w[:, cs].reshape(KD8, P, P).transpose(1, 0, 2))

    in_maps = []
    for c in range(NCORES):
        cs = slice(c * P, (c + 1) * P)
        in_maps.append({
            "xT": xTc,
            "wq": wrearr(Wq, cs),
            "wk": wrearr(Wk, cs),
            "wv": wrearr(Wv, cs),
            "bq": np.ascontiguousarray(bq[cs].reshape(P, 1)),
            "bk": np.ascontiguousarray(bk[cs].reshape(P, 1)),
            "bv": np.ascontiguousarray(bv[cs].reshape(P, 1)),
            "wp": np.ascontiguousarray(Wp[cs, :]),
            "maskb": mb,
            "cmask": cm,
        })
    return in_maps


def run(inputs, trace=False, tmpdir=None):
    """Compile (cached) + run on 8 cores. Returns (output, BassKernelResults)."""
    from concourse import bass_utils
    nc = _get_nc()
    in_maps = make_in_maps(**inputs)
    kwargs = {}
    if trace:
        kwargs = dict(trace=True, tmpdir=tmpdir)
    res = bass_utils.run_bass_kernel_spmd(
        nc, in_maps, core_ids=list(range(NCORES)), **kwargs)
    acc = np.zeros((BS, D), dtype=np.float32)
    for r in res.results:
        acc += r["yp"].astype(np.float32)
    out = acc + np.asarray(inputs["bp"], np.float32)[None, :]
    return out.reshape(B, S, D), res


def kernel(**inputs) -> np.ndarray:
    out, _ = run(inputs, trace=False)
    return out


# revision 46
# speedup vs baseline: 1.0167x; 1.0165x over previous
"""MultiHeadAttention (B=4, S=2048, D=1024, H=16, causal + key mask) on 8 trn2 cores.

Sharding: Megatron-style tensor parallel over heads. Each core owns 2 heads:
column slices of Wq/Wk/Wv (D x 128), the matching row slice of Wp (128 x D).
Each core computes a partial output y_c = attn_c @ Wp_c; host sums the 8
partials and adds bp.

v10 (340986 -> 320096 ns; engine rebalance + pipeline/boundary fixes):
  - ScalarE is pure exp (strict-FIFO queue carries nothing else, so no
    head-of-line hazard): q/k/v projection evictions moved to DVE
    tensor_scalar_add (bias fused, f32 psum -> bf16 sbuf in one op); ALL
    output-projection evictions on DVE.
  - Uniform pipelined schedule: proj chunks run 1 group ahead of the
    attention that consumes them (no dead 22us batch-0 proj phase; chunks
    13-15 double as batch-3 PE filler); exp stream starts ~8us in.
  - Each group's first two score-pairs are emitted under
    tc.high_priority(offset=60) so they jump past the boundary proj/outproj
    bursts in the PE queue and the exp stream restarts promptly.
  - Diagonal blocks: only the 128-col triangular strip is mask-multiplied
    (DVE); PV is split so the wide clean columns chain exp->PV directly.
  - Normalize: f32 reciprocal_approx_fast (DVE) -> partition-broadcast via
    gpsimd DMA descriptors + muls on the otherwise-idle GpSimd (zero PE/DVE
    cost; 6-10us DMA latency hidden by the outproj drain lag). The last two
    groups instead use a short chain: bf16 row cast -> K=1 bf16 broadcast
    matmul (213ns) -> DVE muls off PSUM; the final group also skips the
    at_sb partition-shift DMA by feeding a split output projection (two
    k=64 accumulating matmuls, Wp rows 64-127 re-homed at partitions 0-63).
  - Deferred outproj drain table pushes output projections into the
    ACT-bound batch-3 windows.
  - Host-side relayouts: xT pre-chunked [128, 16, 8, 512] and weights
    [128, 8, 128] (contiguous >=2KB DMA lines; queue busy 200us -> 105us).
  - Exp ACT table preloaded with a dummy activation at t~0.
  - v3 keeps: bf16 matmuls, row-tiled concurrent score pairs, one-block
    score lookahead, PV ones-column denominator trick.

Measured bottleneck structure (per core): PE busy ~268us (true work ~210 +
LDWEIGHTS/sem exposure), ScalarE exp 152us, DVE ~196us, wall 320us. PSUM is
the hard wall (8 banks: scores 2x2 + PV accum 2 + proj/transient 2) -- it
caps score lookahead at 2 and blocks every deeper-pipelining variant tried.
"""

import numpy as np

P = 128
B, S, D, H = 4, 2048, 1024, 16
HD = D // H  # 64
NCORES = 8
HPC = H // NCORES  # 2 heads per core
BS = B * S  # 8192
NB = S // P  # 16 k-blocks per batch
NG = S // 512  # 4 q-groups per batch
NCHUNK = BS // 512  # 16 token chunks

_CACHE = {}


def _build_nc():
    import concourse.mybir as mybir
    from concourse import bacc
    from concourse.tile import TileContext
    from concourse.masks import make_identity
    from contextlib import ExitStack

    f32 = mybir.dt.float32
    bf16 = mybir.dt.bfloat16
    AF = mybir.ActivationFunctionType

    nc = bacc.Bacc("TRN2", target_bir_lowering=False, debug=False,
                   num_devices=NCORES)

    KD = D // P  # 8 contraction chunks
    # pre-chunked x^T: [p, chunk, o, m] = x^T[o*128+p, chunk*512+m]
    xT_d = nc.dram_tensor("xT", [P, NCHUNK, KD, 512], bf16,
                          kind="ExternalInput").ap()
    # weights pre-arranged [p, o, m] = W[o*128+p, m] (contiguous 2KB lines)
    wq_d = nc.dram_tensor("wq", [P, KD, P], bf16, kind="ExternalInput").ap()
    wk_d = nc.dram_tensor("wk", [P, KD, P], bf16, kind="ExternalInput").ap()
    wv_d = nc.dram_tensor("wv", [P, KD, P], bf16, kind="ExternalInput").ap()
    bq_d = nc.dram_tensor("bq", [P, 1], f32, kind="ExternalInput").ap()
    bk_d = nc.dram_tensor("bk", [P, 1], f32, kind="ExternalInput").ap()
    bv_d = nc.dram_tensor("bv", [P, 1], f32, kind="ExternalInput").ap()
    wp_d = nc.dram_tensor("wp", [P, D], bf16, kind="ExternalInput").ap()
    mb_d = nc.dram_tensor("maskb", [P, B * NB], f32, kind="ExternalInput").ap()
    cm_d = nc.dram_tensor("cmask", [P, 4, HPC, 512], bf16,
                          kind="ExternalInput").ap()
    yp_d = nc.dram_tensor("yp", [BS, D], bf16, kind="ExternalOutput").ap()

    with TileContext(nc) as tc:
        with ExitStack() as ctx:
            consts = ctx.enter_context(tc.tile_pool(name="consts", bufs=1))
            big = ctx.enter_context(tc.tile_pool(name="big", bufs=1))
            xpool = ctx.enter_context(tc.tile_pool(name="xpool", bufs=3))
            vtpool = ctx.enter_context(tc.tile_pool(name="vtpool", bufs=2))
            ptpool = ctx.enter_context(tc.tile_pool(name="ptpool", bufs=4))
            pvspool = ctx.enter_context(tc.tile_pool(name="pvs", bufs=2))
            npool = ctx.enter_context(tc.tile_pool(name="npool", bufs=3))
            ypool = ctx.enter_context(tc.tile_pool(name="ypool", bufs=6))
            psum = ctx.enter_context(
                tc.tile_pool(name="psum", bufs=2, space="PSUM"))
            sc2pool = ctx.enter_context(
                tc.tile_pool(name="sc2pool", bufs=2, space="PSUM"))
            pvpool = ctx.enter_context(
                tc.tile_pool(name="pvpool", bufs=2, space="PSUM"))

            # ---- constants (critical path first: wq/bq gate proj chunk 0)
            wq_sb = consts.tile([P, KD, P], bf16, tag="wq")
            wk_sb = consts.tile([P, KD, P], bf16, tag="wk")
            wv_sb = consts.tile([P, KD, P], bf16, tag="wv")
            bq_sb = consts.tile([P, 1], f32, tag="bq")
            bk_sb = consts.tile([P, 1], f32, tag="bk")
            bv_sb = consts.tile([P, 1], f32, tag="bv")
            nc.sync.dma_start(wq_sb[:], wq_d)
            nc.sync.dma_start(bq_sb[:], bq_d)
            nc.sync.dma_start(wk_sb[:], wk_d)
            nc.sync.dma_start(bk_sb[:], bk_d)
            nc.sync.dma_start(wv_sb[:], wv_d)
            nc.sync.dma_start(bv_sb[:], bv_d)
            wp_sb = consts.tile([P, D], bf16, tag="wp")
            nc.sync.dma_start(wp_sb[:], wp_d)
            # rows 64..127 of Wp re-homed at partitions 0..63 for the final
            # group's split output projection (lhsT/rhs base partitions must
            # match)
            wp2_sb = consts.tile([HD, D], bf16, tag="wp2")
            nc.sync.dma_start(wp2_sb[:], wp_d[HD:P, :])
            mb_sb = consts.tile([P, B * NB], f32, tag="mb")
            nc.sync.dma_start(mb_sb[:], mb_d)
            # multiplicative causal masks, [p, j, head, q'] 0/1 bf16
            cm_sb = consts.tile([P, 4, HPC, 512], bf16, tag="cm")
            nc.sync.dma_start(cm_sb[:], cm_d)
            ident = consts.tile([P, P], bf16, tag="ident")
            make_identity(nc, ident[:])
            # ones row on partition 64 (same partition as the PV denominator
            # row) -- bf16 lhsT of the reciprocal-broadcast matmul
            ones64 = consts.tile([P, HD], bf16, tag="ones64")
            nc.vector.memset(ones64[HD:HD + 1, :], 1.0)
            # scratch for the exp ACT-table preload
            warm = consts.tile([P, 1], f32, tag="warm")
            nc.scalar.activation(warm[:], bq_sb[:], AF.Exp)

            # ---- persistent activations (all bf16) ----
            qt_sb = big.tile([P, B, S], bf16, tag="qt")  # Q^T
            kt_sb = big.tile([P, B, S], bf16, tag="kt")  # K^T
            # V in [s, hd] layout + ones col: [p=s%128, b, sblock, h, 65]
            v_sb = big.tile([P, B, NB, HPC, HD + 1], bf16, tag="v")
            at_sb = big.tile([P, B, S], bf16, tag="at")  # attn^T (normalized)
            nc.vector.memset(v_sb[:, :, :, :, HD], 1.0)

            # ---- projections for one 512-row chunk of x ----
            def proj_chunk(c):
                b, sc = divmod(c, NG)
                xt = xpool.tile([P, KD, 512], bf16, tag="xt")
                # gpsimd DMA queue: the sync queue carries the y writes whose
                # in-queue semaphore waits would head-of-line block this load
                nc.gpsimd.dma_start(xt[:], xT_d[:, c, :, :])
                ssl = slice(sc * 512, (sc + 1) * 512)
                for which in range(3):
                    w_sb = (wq_sb, wk_sb, wv_sb)[which]
                    ps = psum.tile([P, 512], f32, tag="ps")
                    for o in range(KD):
                        nc.tensor.matmul(
                            ps[:], lhsT=w_sb[:, o, :], rhs=xt[:, o, :],
                            start=(o == 0), stop=(o == KD - 1))
                    if which == 0:
                        nc.vector.tensor_scalar_add(qt_sb[:, b, ssl], ps[:],
                                                    bq_sb[:])
                    elif which == 1:
                        nc.vector.tensor_scalar_add(kt_sb[:, b, ssl], ps[:],
                                                    bk_sb[:])
                    else:
                        vt = vtpool.tile([P, 512], bf16, tag="vt")
                        nc.vector.tensor_scalar_add(vt[:], ps[:], bv_sb[:])
                        for t in range(4):
                            # shares the "ps" slots (pools size per tag)
                            trp = psum.tile([P, P], bf16, tag="ps")
                            nc.tensor.transpose(
                                trp[:], vt[:, t * P:(t + 1) * P], ident[:])
                            sb_i = sc * 4 + t
                            nc.vector.tensor_copy(
                                v_sb[:, b, sb_i, 0, 0:HD], trp[:, 0:HD])
                            nc.vector.tensor_copy(
                                v_sb[:, b, sb_i, 1, 0:HD],
                                trp[:, HD:2 * HD])

            # ---- output projection for one (b, g) q-group ----
            def outproj(b, g):
                for qc in range(4):
                    q0 = g * 512 + qc * P
                    r0 = b * S + q0
                    y_sb = ypool.tile([P, 2, 512], bf16, tag="y",
                                      name=f"y_{b}_{g}_{qc}")
                    for half in range(2):
                        yp_ps = psum.tile([P, 512], f32, tag="ps",
                                          name=f"yps_{b}_{g}_{qc}_{half}")
                        nc.tensor.matmul(
                            yp_ps[:],
                            lhsT=at_sb[:, b, q0:q0 + P],
                            rhs=wp_sb[:, half * 512:(half + 1) * 512],
                            start=True, stop=True)
                        # all evictions on DVE: ScalarE is strict FIFO, so
                        # an eviction waiting on its outproj matmul would
                        # head-of-line block the next group's exp
                        nc.vector.tensor_copy(y_sb[:, half, :], yp_ps[:])
                    nc.sync.dma_start(
                        yp_d[r0:r0 + P, :],
                        y_sb[:].rearrange("p a n -> p (a n)"))

            # ---- attention for one (b, g) q-group ----
            def attn_group(b, g, pending, split_out=False, tail=False):
                gsl = slice(g * 512, (g + 1) * 512)
                nkb = 4 * (g + 1)
                order = list(range(nkb))
                pvs = [pvpool.tile([P, 512], f32, tag="pv",
                                   name=f"pv_{b}_{g}_{h}")
                       for h in range(HPC)]

                def scores(kb):
                    j = kb - 4 * g
                    # diagonal blocks: q < 128*j is fully masked
                    qo = 128 * max(j, 0)
                    sc2 = sc2pool.tile([P, HPC, 512], f32, tag="sc2",
                                       name=f"sc2_{b}_{g}_{kb}")
                    for h in range(HPC):
                        hsl = slice(h * HD, (h + 1) * HD)
                        nc.tensor.matmul(
                            sc2[:, h, qo:512],
                            lhsT=kt_sb[hsl, b, kb * P:(kb + 1) * P],
                            rhs=qt_sb[hsl, b, g * 512 + qo:(g + 1) * 512],
                            start=True, stop=True)
                    return sc2, kb, j, qo

                # hoist this group's first two score-pairs past the
                # boundary proj/outproj bursts (priority = emission order,
                # offset 60 ~ one proj chunk + drain) so the exp stream
                # restarts immediately at group boundaries
                with tc.high_priority(offset=60):
                    cur = scores(order[0])
                for i, kb in enumerate(order):
                    if i + 1 < nkb:
                        if i == 0:
                            with tc.high_priority(offset=60):
                                nxt = scores(order[1])
                        else:
                            nxt = scores(order[i + 1])
                    else:
                        nxt = None
                    sc2, _, j, qo = cur
                    col = b * NB + kb
                    pt = ptpool.tile([P, HPC, 512], bf16, tag="pt")
                    if qo == 0:
                        nc.scalar.activation(pt[:], sc2[:], AF.Exp,
                                             bias=mb_sb[:, col:col + 1])
                    else:
                        nc.scalar.activation(pt[:, :, qo:512],
                                             sc2[:, :, qo:512], AF.Exp,
                                             bias=mb_sb[:, col:col + 1])
                    if j >= 0:
                        # causal mask: only the 128-col strip [qo, qo+128)
                        # is triangular -- mask just that strip so the wide
                        # clean part of PV never waits on the mask-mul
                        nc.vector.tensor_mul(pt[:, :, qo:qo + P],
                                             pt[:, :, qo:qo + P],
                                             cm_sb[:, j, :, qo:qo + P])
                    for h in range(HPC):
                        if j >= 0 and qo + P < 512:
                            # clean columns: chain is exp -> PV directly
                            nc.tensor.matmul(
                                pvs[h][0:HD + 1, qo + P:512],
                                lhsT=v_sb[:, b, kb, h, :],
                                rhs=pt[:, h, qo + P:512],
                                start=(i == 0), stop=False)
                            nc.tensor.matmul(
                                pvs[h][0:HD + 1, qo:qo + P],
                                lhsT=v_sb[:, b, kb, h, :],
                                rhs=pt[:, h, qo:qo + P],
                                start=False, stop=(i == nkb - 1))
                        else:
                            nc.tensor.matmul(
                                pvs[h][0:HD + 1, qo:512],
                                lhsT=v_sb[:, b, kb, h, :],
                                rhs=pt[:, h, qo:512],
                                start=(i == 0), stop=(i == nkb - 1))
                    cur = nxt
                # evict PV psums immediately so the banks recycle without
                # waiting on the normalize chain
                pvs_sb = pvspool.tile([P, HPC, 512], f32, tag="pvs")
                for h in range(HPC):
                    nc.vector.tensor_copy(pvs_sb[0:HD + 1, h, :],
                                          pvs[h][0:HD + 1, :])
                if not split_out:
                    pending.append((b, g))
                # ---- normalize: 1/denom (row 64, bf16) -> K=1 bf16 matmul
                # broadcast into PSUM -> DVE muls reading PSUM ----
                # reciprocal_approx_fast misbehaves on single-partition
                # slices -- run it over the full tile (unused rows discarded)
                rcp = npool.tile([P, HPC, 512], f32, tag="rcp")
                nc.vector.reciprocal_approx_fast(rcp[:], pvs_sb[:])
                if not tail:
                    # slack-tolerant path (outproj lag hides the 6-10us DMA
                    # latency): partition-broadcast via DMA descriptors and
                    # muls on the otherwise-idle GpSimd -- zero PE cost and
                    # near-zero DVE cost
                    tmp = npool.tile([HD, 512], bf16, tag="tmp")
                    dbc = npool.tile([HD, HPC, 512], f32, tag="dbc")
                    for h in range(HPC):
                        nc.gpsimd.dma_start(
                            dbc[:, h, :],
                            rcp[HD:HD + 1, h, None, :]
                            .to_broadcast((1, HD, 512)))
                    nc.gpsimd.tensor_mul(at_sb[0:HD, b, gsl],
                                         pvs_sb[0:HD, 0, :], dbc[:, 0, :])
                    nc.gpsimd.tensor_mul(tmp[:], pvs_sb[0:HD, 1, :],
                                         dbc[:, 1, :])
                    nc.gpsimd.dma_start(at_sb[HD:2 * HD, b, gsl], tmp[:])
                    return None
                # tail groups: short matmul-broadcast chain
                # bf16 copy of the denominator-reciprocal row: keeps the
                # broadcast matmul at 1 cyc/row (fp32 rhs would be 4x)
                rcpb = npool.tile([P, HPC, 512], bf16, tag="rcpb")
                nc.vector.tensor_copy(rcpb[HD:HD + 1, :, :],
                                      rcp[HD:HD + 1, :, :])
                bc = [pvpool.tile([P, 512], f32, tag="pv",
                                  name=f"bc_{b}_{g}_{h}")
                      for h in range(HPC)]
                for h in range(HPC):
                    nc.tensor.matmul(
                        bc[h][0:HD, :], lhsT=ones64[HD:HD + 1, :],
                        rhs=rcpb[HD:HD + 1, h, :], start=True, stop=True)
                tmp = npool.tile([HD, 512], bf16, tag="tmp")
                if split_out:
                    # final group: keep both halves at partitions 0..63 and
                    # feed the split output projection directly -- no at_sb
                    # partition-shift DMA in the tail chain
                    a0 = npool.tile([HD, 512], bf16, tag="a0")
                    nc.vector.tensor_mul(a0[:], pvs_sb[0:HD, 0, :],
                                         bc[0][0:HD, :])
                    nc.vector.tensor_mul(tmp[:], pvs_sb[0:HD, 1, :],
                                         bc[1][0:HD, :])
                    return a0, tmp
                nc.vector.tensor_mul(at_sb[0:HD, b, gsl],
                                     pvs_sb[0:HD, 0, :], bc[0][0:HD, :])
                nc.vector.tensor_mul(tmp[:], pvs_sb[0:HD, 1, :],
                                     bc[1][0:HD, :])
                nc.gpsimd.dma_start(at_sb[HD:2 * HD, b, gsl], tmp[:])
                return None

            # ---- split output projection for the final group: two k=64
            # accumulating matmuls per psum, lhsT halves at partitions 0-63
            def outproj_split(b, g, a0, a1):
                for qc in range(4):
                    q0 = g * 512 + qc * P
                    r0 = b * S + q0
                    qsl = slice(qc * P, (qc + 1) * P)
                    y_sb = ypool.tile([P, 2, 512], bf16, tag="y",
                                      name=f"ys_{b}_{g}_{qc}")
                    for half in range(2):
                        nsl = slice(half * 512, (half + 1) * 512)
                        yp_ps = psum.tile([P, 512], f32, tag="ps",
                                          name=f"yss_{b}_{g}_{qc}_{half}")
                        nc.tensor.matmul(
                            yp_ps[:], lhsT=a0[:, qsl],
                            rhs=wp_sb[0:HD, nsl], start=True, stop=False)
                        nc.tensor.matmul(
                            yp_ps[:], lhsT=a1[:, qsl],
                            rhs=wp2_sb[:, nsl], start=False, stop=True)
                        nc.vector.tensor_copy(y_sb[:, half, :], yp_ps[:])
                    nc.sync.dma_start(
                        yp_d[r0:r0 + P, :],
                        y_sb[:].rearrange("p a n -> p (a n)"))

            # ---- schedule: uniform pipeline. proj chunks run 2 groups ahead
            # of the attention that consumes them, so batch-0 attention (and
            # its exp stream) starts ~8us in instead of after a dead 22us
            # proj-only phase. Output projections are deferred into the
            # ACT-bound batch-3 window via the drain table. ----
            DRAIN = [[0, 0, 1, 1], [0, 1, 1, 1], [0, 0, 1, 1], [2, 2, 2, 9]]
            proj_chunk(0)
            pending = []
            for b in range(B):
                for g in range(NG):
                    last = (b == B - 1 and g == NG - 1)
                    ret = attn_group(b, g, pending, split_out=last,
                                     tail=(b == B - 1 and g >= NG - 2))
                    for _ in range(DRAIN[b][g]):
                        if pending:
                            outproj(*pending.pop(0))
                    if last:
                        outproj_split(b, g, *ret)
                    # 1-ahead proj cadence: keeps chunks 13-15 as batch-3
                    # PE filler while staying a full group ahead of use
                    c = NG * b + g + 1
                    if c < NCHUNK:
                        proj_chunk(c)

    nc.compile()
    return nc


def _get_nc():
    if "nc" not in _CACHE:
        _CACHE["nc"] = _build_nc()
    return _CACHE["nc"]


def make_in_maps(x, attention_mask, Wq, bq, Wk, bk, Wv, bv, Wp, bp):
    """Host-side sharding: build the 8 per-core device input maps."""
    import ml_dtypes
    bf16 = ml_dtypes.bfloat16
    KD8 = D // P
    x = np.asarray(x, dtype=np.float32)
    scale = np.float32(1.0 / np.sqrt(HD))
    xT = x.reshape(BS, D).T.astype(bf16)  # [D, BS]
    # pre-chunked layout: [p, chunk, o, m] = xT[o*128+p, chunk*512+m]
    xTc = np.ascontiguousarray(
        xT.reshape(D // P, P, NCHUNK, 512).transpose(1, 2, 0, 3))
    mb = (np.asarray(attention_mask).astype(np.float32) - 1.0) * np.float32(1e9)
    mb = np.ascontiguousarray(
        mb.reshape(B, NB, P).transpose(2, 0, 1).reshape(P, B * NB))
    # multiplicative causal masks: 1 where 128*j + p <= q', else 0;
    # duplicated for the two heads: [128, 4, 2, 512]
    pp = np.arange(P)[:, None]
    ff = np.arange(512)[None, :]
    cm = np.stack(
        [np.where(P * j + pp <= ff, 1.0, 0.0).astype(bf16)
         for j in range(4)], axis=1)  # [128, 4, 512]
    cm = np.ascontiguousarray(
        np.broadcast_to(cm[:, :, None, :], (P, 4, HPC, 512)))

    Wq = (np.asarray(Wq, np.float32) * scale).astype(bf16)
    bq = np.asarray(bq, np.float32) * scale
    Wk = np.asarray(Wk, np.float32).astype(bf16)
    bk = np.asarray(bk, np.float32)
    Wv = np.asarray(Wv, np.float32).astype(bf16)
    bv = np.asarray(bv, np.float32)
    Wp = np.asarray(Wp, np.float32).astype(bf16)

    def wrearr(w, cs):
        # [1024, 128] core slice -> [p, o, m] = W[o*128+p, m]
        return np.ascontiguousarray(
            w[:, cs].reshape(KD8, P, P).transpose(1, 0, 2))

    in_maps = []
    for c in range(NCORES):
        cs = slice(c * P, (c + 1) * P)
        in_maps.append({
            "xT": xTc,
            "wq": wrearr(Wq, cs),
            "wk": wrearr(Wk, cs),
            "wv": wrearr(Wv, cs),
            "bq": np.ascontiguousarray(bq[cs].reshape(P, 1)),
            "bk": np.ascontiguousarray(bk[cs].reshape(P, 1)),
            "bv": np.ascontiguousarray(bv[cs].reshape(P, 1)),
            "wp": np.ascontiguousarray(Wp[cs, :]),
            "maskb": mb,
            "cmask": cm,
        })
    return in_maps


def run(inputs, trace=False, tmpdir=None):
    """Compile (cached) + run on 8 cores. Returns (output, BassKernelResults)."""
    from concourse import bass_utils
    nc = _get_nc()
    in_maps = make_in_maps(**inputs)
    kwargs = {}
    if trace:
        kwargs = dict(trace=True, tmpdir=tmpdir)
    res = bass_utils.run_bass_kernel_spmd(
        nc, in_maps, core_ids=list(range(NCORES)), **kwargs)
    acc = np.zeros((BS, D), dtype=np.float32)
    for r in res.results:
        acc += r["yp"].astype(np.float32)
    out = acc + np.asarray(inputs["bp"], np.float32)[None, :]
    return out.reshape(B, S, D), res


def kernel(**inputs) -> np.ndarray:
    out, _ = run(inputs, trace=False)
    return out


# revision 50
# speedup vs baseline: 1.0175x; 1.0008x over previous
"""MultiHeadAttention (B=4, S=2048, D=1024, H=16, causal + key mask) on 8 trn2 cores.

Sharding: Megatron-style tensor parallel over heads. Each core owns 2 heads:
column slices of Wq/Wk/Wv (D x 128), the matching row slice of Wp (128 x D).
Each core computes a partial output y_c = attn_c @ Wp_c; host sums the 8
partials and adds bp.

v10 (340986 -> 320096 ns; engine rebalance + pipeline/boundary fixes):
  - ScalarE is pure exp (strict-FIFO queue carries nothing else, so no
    head-of-line hazard): q/k/v projection evictions moved to DVE
    tensor_scalar_add (bias fused, f32 psum -> bf16 sbuf in one op); ALL
    output-projection evictions on DVE.
  - Uniform pipelined schedule: proj chunks run 1 group ahead of the
    attention that consumes them (no dead 22us batch-0 proj phase; chunks
    13-15 double as batch-3 PE filler); exp stream starts ~8us in.
  - Each group's first two score-pairs are emitted under
    tc.high_priority(offset=60) so they jump past the boundary proj/outproj
    bursts in the PE queue and the exp stream restarts promptly.
  - Diagonal blocks: only the 128-col triangular strip is mask-multiplied
    (DVE); PV is split so the wide clean columns chain exp->PV directly.
  - Normalize: f32 reciprocal_approx_fast (DVE) -> partition-broadcast via
    gpsimd DMA descriptors + muls on the otherwise-idle GpSimd (zero PE/DVE
    cost; 6-10us DMA latency hidden by the outproj drain lag). The last two
    groups instead use a short chain: bf16 row cast -> K=1 bf16 broadcast
    matmul (213ns) -> DVE muls off PSUM; the final group also skips the
    at_sb partition-shift DMA by feeding a split output projection (two
    k=64 accumulating matmuls, Wp rows 64-127 re-homed at partitions 0-63).
  - Deferred outproj drain table pushes output projections into the
    ACT-bound batch-3 windows.
  - Host-side relayouts: xT pre-chunked [128, 16, 8, 512] and weights
    [128, 8, 128] (contiguous >=2KB DMA lines; queue busy 200us -> 105us).
  - Exp ACT table preloaded with a dummy activation at t~0.
  - v3 keeps: bf16 matmuls, row-tiled concurrent score pairs, one-block
    score lookahead, PV ones-column denominator trick.

Measured bottleneck structure (per core): PE busy ~268us (true work ~210 +
LDWEIGHTS/sem exposure), ScalarE exp 152us, DVE ~196us, wall 320us. PSUM is
the hard wall (8 banks: scores 2x2 + PV accum 2 + proj/transient 2) -- it
caps score lookahead at 2 and blocks every deeper-pipelining variant tried.
"""

import numpy as np

P = 128
B, S, D, H = 4, 2048, 1024, 16
HD = D // H  # 64
NCORES = 8
HPC = H // NCORES  # 2 heads per core
BS = B * S  # 8192
NB = S // P  # 16 k-blocks per batch
NG = S // 512  # 4 q-groups per batch
NCHUNK = BS // 512  # 16 token chunks

_CACHE = {}


def _build_nc():
    import concourse.mybir as mybir
    from concourse import bacc
    from concourse.tile import TileContext
    from concourse.masks import make_identity
    from contextlib import ExitStack

    f32 = mybir.dt.float32
    bf16 = mybir.dt.bfloat16
    AF = mybir.ActivationFunctionType

    nc = bacc.Bacc("TRN2", target_bir_lowering=False, debug=False,
                   num_devices=NCORES)

    KD = D // P  # 8 contraction chunks
    # pre-chunked x^T: [p, chunk, o, m] = x^T[o*128+p, chunk*512+m]
    xT_d = nc.dram_tensor("xT", [P, NCHUNK, KD, 512], bf16,
                          kind="ExternalInput").ap()
    # weights pre-arranged [p, o, m] = W[o*128+p, m] (contiguous 2KB lines)
    wq_d = nc.dram_tensor("wq", [P, KD, P], bf16, kind="ExternalInput").ap()
    wk_d = nc.dram_tensor("wk", [P, KD, P], bf16, kind="ExternalInput").ap()
    wv_d = nc.dram_tensor("wv", [P, KD, P], bf16, kind="ExternalInput").ap()
    bq_d = nc.dram_tensor("bq", [P, 1], f32, kind="ExternalInput").ap()
    bk_d = nc.dram_tensor("bk", [P, 1], f32, kind="ExternalInput").ap()
    bv_d = nc.dram_tensor("bv", [P, 1], f32, kind="ExternalInput").ap()
    wp_d = nc.dram_tensor("wp", [P, D], bf16, kind="ExternalInput").ap()
    mb_d = nc.dram_tensor("maskb", [P, B * NB], f32, kind="ExternalInput").ap()
    cm_d = nc.dram_tensor("cmask", [P, 4, HPC, 512], bf16,
                          kind="ExternalInput").ap()
    yp_d = nc.dram_tensor("yp", [BS, D], bf16, kind="ExternalOutput").ap()

    with TileContext(nc) as tc:
        with ExitStack() as ctx:
            consts = ctx.enter_context(tc.tile_pool(name="consts", bufs=1))
            big = ctx.enter_context(tc.tile_pool(name="big", bufs=1))
            xpool = ctx.enter_context(tc.tile_pool(name="xpool", bufs=3))
            vtpool = ctx.enter_context(tc.tile_pool(name="vtpool", bufs=2))
            ptpool = ctx.enter_context(tc.tile_pool(name="ptpool", bufs=4))
            pvspool = ctx.enter_context(tc.tile_pool(name="pvs", bufs=2))
            npool = ctx.enter_context(tc.tile_pool(name="npool", bufs=3))
            ypool = ctx.enter_context(tc.tile_pool(name="ypool", bufs=6))
            psum = ctx.enter_context(
                tc.tile_pool(name="psum", bufs=2, space="PSUM"))
            sc2pool = ctx.enter_context(
                tc.tile_pool(name="sc2pool", bufs=2, space="PSUM"))
            pvpool = ctx.enter_context(
                tc.tile_pool(name="pvpool", bufs=2, space="PSUM"))

            # ---- constants (critical path first: wq/bq gate proj chunk 0)
            wq_sb = consts.tile([P, KD, P], bf16, tag="wq")
            wk_sb = consts.tile([P, KD, P], bf16, tag="wk")
            wv_sb = consts.tile([P, KD, P], bf16, tag="wv")
            bq_sb = consts.tile([P, 1], f32, tag="bq")
            bk_sb = consts.tile([P, 1], f32, tag="bk")
            bv_sb = consts.tile([P, 1], f32, tag="bv")
            nc.sync.dma_start(wq_sb[:], wq_d)
            nc.sync.dma_start(bq_sb[:], bq_d)
            nc.sync.dma_start(wk_sb[:], wk_d)
            nc.sync.dma_start(bk_sb[:], bk_d)
            nc.sync.dma_start(wv_sb[:], wv_d)
            nc.sync.dma_start(bv_sb[:], bv_d)
            wp_sb = consts.tile([P, D], bf16, tag="wp")
            nc.sync.dma_start(wp_sb[:], wp_d)
            # rows 64..127 of Wp re-homed at partitions 0..63 for the final
            # group's split output projection (lhsT/rhs base partitions must
            # match)
            wp2_sb = consts.tile([HD, D], bf16, tag="wp2")
            nc.sync.dma_start(wp2_sb[:], wp_d[HD:P, :])
            mb_sb = consts.tile([P, B * NB], f32, tag="mb")
            nc.sync.dma_start(mb_sb[:], mb_d)
            # multiplicative causal masks, [p, j, head, q'] 0/1 bf16
            cm_sb = consts.tile([P, 4, HPC, 512], bf16, tag="cm")
            nc.sync.dma_start(cm_sb[:], cm_d)
            ident = consts.tile([P, P], bf16, tag="ident")
            make_identity(nc, ident[:])
            # ones row on partition 64 (same partition as the PV denominator
            # row) -- bf16 lhsT of the reciprocal-broadcast matmul
            ones64 = consts.tile([P, HD], bf16, tag="ones64")
            nc.vector.memset(ones64[HD:HD + 1, :], 1.0)
            # scratch for the exp ACT-table preload
            warm = consts.tile([P, 1], f32, tag="warm")
            nc.scalar.activation(warm[:], bq_sb[:], AF.Exp)

            # ---- persistent activations (all bf16) ----
            qt_sb = big.tile([P, B, S], bf16, tag="qt")  # Q^T
            kt_sb = big.tile([P, B, S], bf16, tag="kt")  # K^T
            # V in [s, hd] layout + ones col: [p=s%128, b, sblock, h, 65]
            v_sb = big.tile([P, B, NB, HPC, HD + 1], bf16, tag="v")
            at_sb = big.tile([P, B, S], bf16, tag="at")  # attn^T (normalized)
            nc.vector.memset(v_sb[:, :, :, :, HD], 1.0)

            # ---- projections for one 512-row chunk of x ----
            def proj_chunk(c):
                b, sc = divmod(c, NG)
                xt = xpool.tile([P, KD, 512], bf16, tag="xt")
                # gpsimd DMA queue: the sync queue carries the y writes whose
                # in-queue semaphore waits would head-of-line block this load
                nc.gpsimd.dma_start(xt[:], xT_d[:, c, :, :])
                ssl = slice(sc * 512, (sc + 1) * 512)
                for which in range(3):
                    w_sb = (wq_sb, wk_sb, wv_sb)[which]
                    ps = psum.tile([P, 512], f32, tag="ps")
                    for o in range(KD):
                        nc.tensor.matmul(
                            ps[:], lhsT=w_sb[:, o, :], rhs=xt[:, o, :],
                            start=(o == 0), stop=(o == KD - 1))
                    if which == 0:
                        nc.vector.tensor_scalar_add(qt_sb[:, b, ssl], ps[:],
                                                    bq_sb[:])
                    elif which == 1:
                        nc.vector.tensor_scalar_add(kt_sb[:, b, ssl], ps[:],
                                                    bk_sb[:])
                    else:
                        vt = vtpool.tile([P, 512], bf16, tag="vt")
                        nc.vector.tensor_scalar_add(vt[:], ps[:], bv_sb[:])
                        for t in range(4):
                            # shares the "ps" slots (pools size per tag)
                            trp = psum.tile([P, P], bf16, tag="ps")
                            nc.tensor.transpose(
                                trp[:], vt[:, t * P:(t + 1) * P], ident[:])
                            sb_i = sc * 4 + t
                            nc.vector.tensor_copy(
                                v_sb[:, b, sb_i, 0, 0:HD], trp[:, 0:HD])
                            nc.vector.tensor_copy(
                                v_sb[:, b, sb_i, 1, 0:HD],
                                trp[:, HD:2 * HD])

            # ---- output projection for one (b, g) q-group ----
            def outproj(b, g):
                for qc in range(4):
                    q0 = g * 512 + qc * P
                    r0 = b * S + q0
                    y_sb = ypool.tile([P, 2, 512], bf16, tag="y",
                                      name=f"y_{b}_{g}_{qc}")
                    for half in range(2):
                        yp_ps = psum.tile([P, 512], f32, tag="ps",
                                          name=f"yps_{b}_{g}_{qc}_{half}")
                        nc.tensor.matmul(
                            yp_ps[:],
                            lhsT=at_sb[:, b, q0:q0 + P],
                            rhs=wp_sb[:, half * 512:(half + 1) * 512],
                            start=True, stop=True)
                        # all evictions on DVE: ScalarE is strict FIFO, so
                        # an eviction waiting on its outproj matmul would
                        # head-of-line block the next group's exp
                        nc.vector.tensor_copy(y_sb[:, half, :], yp_ps[:])
                    nc.sync.dma_start(
                        yp_d[r0:r0 + P, :],
                        y_sb[:].rearrange("p a n -> p (a n)"))

            # scores for one (b, g, kb): row-tiled concurrent pair
            def scores_for(b, g, kb):
                j = kb - 4 * g
                # diagonal blocks: q < 128*j is fully masked
                qo = 128 * max(j, 0)
                sc2 = sc2pool.tile([P, HPC, 512], f32, tag="sc2",
                                   name=f"sc2_{b}_{g}_{kb}")
                for h in range(HPC):
                    hsl = slice(h * HD, (h + 1) * HD)
                    nc.tensor.matmul(
                        sc2[:, h, qo:512],
                        lhsT=kt_sb[hsl, b, kb * P:(kb + 1) * P],
                        rhs=qt_sb[hsl, b, g * 512 + qo:(g + 1) * 512],
                        start=True, stop=True)
                return sc2, kb, j, qo

            # ---- attention for one (b, g) q-group. `first` is this
            # group's pre-emitted first score-pair: the main loop emits it
            # BEFORE the previous boundary's proj/outproj bursts so the exp
            # stream restarts immediately at group boundaries ----
            def attn_group(b, g, pending, split_out=False, tail=False,
                           first=None):
                gsl = slice(g * 512, (g + 1) * 512)
                nkb = 4 * (g + 1)
                order = list(range(nkb))
                pvs = [pvpool.tile([P, 512], f32, tag="pv",
                                   name=f"pv_{b}_{g}_{h}")
                       for h in range(HPC)]

                def scores(kb):
                    return scores_for(b, g, kb)

                cur = first if first is not None else scores(order[0])
                for i, kb in enumerate(order):
                    nxt = scores(order[i + 1]) if i + 1 < nkb else None
                    sc2, _, j, qo = cur
                    col = b * NB + kb
                    pt = ptpool.tile([P, HPC, 512], bf16, tag="pt")
                    if qo == 0:
                        nc.scalar.activation(pt[:], sc2[:], AF.Exp,
                                             bias=mb_sb[:, col:col + 1])
                    else:
                        nc.scalar.activation(pt[:, :, qo:512],
                                             sc2[:, :, qo:512], AF.Exp,
                                             bias=mb_sb[:, col:col + 1])
                    if j >= 0:
                        # causal mask: only the 128-col strip [qo, qo+128)
                        # is triangular -- mask just that strip so the wide
                        # clean part of PV never waits on the mask-mul
                        nc.vector.tensor_mul(pt[:, :, qo:qo + P],
                                             pt[:, :, qo:qo + P],
                                             cm_sb[:, j, :, qo:qo + P])
                    for h in range(HPC):
                        if j >= 0 and qo + P < 512:
                            # clean columns: chain is exp -> PV directly
                            nc.tensor.matmul(
                                pvs[h][0:HD + 1, qo + P:512],
                                lhsT=v_sb[:, b, kb, h, :],
                                rhs=pt[:, h, qo + P:512],
                                start=(i == 0), stop=False)
                            nc.tensor.matmul(
                                pvs[h][0:HD + 1, qo:qo + P],
                                lhsT=v_sb[:, b, kb, h, :],
                                rhs=pt[:, h, qo:qo + P],
                                start=False, stop=(i == nkb - 1))
                        else:
                            nc.tensor.matmul(
                                pvs[h][0:HD + 1, qo:512],
                                lhsT=v_sb[:, b, kb, h, :],
                                rhs=pt[:, h, qo:512],
                                start=(i == 0), stop=(i == nkb - 1))
                    cur = nxt
                # evict PV psums immediately so the banks recycle without
                # waiting on the normalize chain
                pvs_sb = pvspool.tile([P, HPC, 512], f32, tag="pvs")
                for h in range(HPC):
                    nc.vector.tensor_copy(pvs_sb[0:HD + 1, h, :],
                                          pvs[h][0:HD + 1, :])
                if not split_out:
                    pending.append((b, g))
                # ---- normalize: 1/denom (row 64, bf16) -> K=1 bf16 matmul
                # broadcast into PSUM -> DVE muls reading PSUM ----
                # reciprocal_approx_fast misbehaves on single-partition
                # slices -- run it over the full tile (unused rows discarded)
                rcp = npool.tile([P, HPC, 512], f32, tag="rcp")
                nc.vector.reciprocal_approx_fast(rcp[:], pvs_sb[:])
                if not tail:
                    # slack-tolerant path (outproj lag hides the 6-10us DMA
                    # latency): partition-broadcast via DMA descriptors and
                    # muls on the otherwise-idle GpSimd -- zero PE cost and
                    # near-zero DVE cost
                    tmp = npool.tile([HD, 512], bf16, tag="tmp")
                    dbc = npool.tile([HD, HPC, 512], f32, tag="dbc")
                    for h in range(HPC):
                        nc.gpsimd.dma_start(
                            dbc[:, h, :],
                            rcp[HD:HD + 1, h, None, :]
                            .to_broadcast((1, HD, 512)))
                    nc.gpsimd.tensor_mul(at_sb[0:HD, b, gsl],
                                         pvs_sb[0:HD, 0, :], dbc[:, 0, :])
                    nc.gpsimd.tensor_mul(tmp[:], pvs_sb[0:HD, 1, :],
                                         dbc[:, 1, :])
                    nc.gpsimd.dma_start(at_sb[HD:2 * HD, b, gsl], tmp[:])
                    return None
                # tail groups: short matmul-broadcast chain
                # bf16 copy of the denominator-reciprocal row: keeps the
                # broadcast matmul at 1 cyc/row (fp32 rhs would be 4x)
                rcpb = npool.tile([P, HPC, 512], bf16, tag="rcpb")
                nc.vector.tensor_copy(rcpb[HD:HD + 1, :, :],
                                      rcp[HD:HD + 1, :, :])
                bc = [pvpool.tile([P, 512], f32, tag="pv",
                                  name=f"bc_{b}_{g}_{h}")
                      for h in range(HPC)]
                for h in range(HPC):
                    nc.tensor.matmul(
                        bc[h][0:HD, :], lhsT=ones64[HD:HD + 1, :],
                        rhs=rcpb[HD:HD + 1, h, :], start=True, stop=True)
                tmp = npool.tile([HD, 512], bf16, tag="tmp")
                if split_out:
                    # final group: keep both halves at partitions 0..63 and
                    # feed the split output projection directly -- no at_sb
                    # partition-shift DMA in the tail chain
                    a0 = npool.tile([HD, 512], bf16, tag="a0")
                    nc.vector.tensor_mul(a0[:], pvs_sb[0:HD, 0, :],
                                         bc[0][0:HD, :])
                    nc.vector.tensor_mul(tmp[:], pvs_sb[0:HD, 1, :],
                                         bc[1][0:HD, :])
                    return a0, tmp
                nc.vector.tensor_mul(at_sb[0:HD, b, gsl],
                                     pvs_sb[0:HD, 0, :], bc[0][0:HD, :])
                nc.vector.tensor_mul(tmp[:], pvs_sb[0:HD, 1, :],
                                     bc[1][0:HD, :])
                nc.gpsimd.dma_start(at_sb[HD:2 * HD, b, gsl], tmp[:])
                return None

            # ---- split output projection for the final group: two k=64
            # accumulating matmuls per psum, lhsT halves at partitions 0-63
            def outproj_split(b, g, a0, a1):
                for qc in range(4):
                    q0 = g * 512 + qc * P
                    r0 = b * S + q0
                    qsl = slice(qc * P, (qc + 1) * P)
                    y_sb = ypool.tile([P, 2, 512], bf16, tag="y",
                                      name=f"ys_{b}_{g}_{qc}")
                    for half in range(2):
                        nsl = slice(half * 512, (half + 1) * 512)
                        yp_ps = psum.tile([P, 512], f32, tag="ps",
                                          name=f"yss_{b}_{g}_{qc}_{half}")
                        nc.tensor.matmul(
                            yp_ps[:], lhsT=a0[:, qsl],
                            rhs=wp_sb[0:HD, nsl], start=True, stop=False)
                        nc.tensor.matmul(
                            yp_ps[:], lhsT=a1[:, qsl],
                            rhs=wp2_sb[:, nsl], start=False, stop=True)
                        nc.vector.tensor_copy(y_sb[:, half, :], yp_ps[:])
                    nc.sync.dma_start(
                        yp_d[r0:r0 + P, :],
                        y_sb[:].rearrange("p a n -> p (a n)"))

            # ---- schedule: uniform pipeline. proj chunks run 2 groups ahead
            # of the attention that consumes them, so batch-0 attention (and
            # its exp stream) starts ~8us in instead of after a dead 22us
            # proj-only phase. Output projections are deferred into the
            # ACT-bound batch-3 window via the drain table. ----
            DRAIN = [[0, 0, 1, 1], [0, 1, 1, 1], [0, 0, 1, 1], [2, 2, 2, 9]]
            proj_chunk(0)
            proj_chunk(1)
            pending = []
            first = None
            for b in range(B):
                for g in range(NG):
                    last = (b == B - 1 and g == NG - 1)
                    ret = attn_group(b, g, pending, split_out=last,
                                     tail=(b == B - 1 and g >= NG - 2),
                                     first=first)
                    # 2-ahead proj cadence: the chunk feeding group gi+1 was
                    # already emitted at gi-1, so the next group's first
                    # scores have NO dependency on this boundary's proj
                    # burst and the exp stream restarts immediately
                    c = NG * b + g + 2
                    if c < NCHUNK:
                        proj_chunk(c)
                    for _ in range(DRAIN[b][g]):
                        if pending:
                            outproj(*pending.pop(0))
                    if last:
                        outproj_split(b, g, *ret)

    nc.compile()
    return nc


def _get_nc():
    if "nc" not in _CACHE:
        _CACHE["nc"] = _build_nc()
    return _CACHE["nc"]


def make_in_maps(x, attention_mask, Wq, bq, Wk, bk, Wv, bv, Wp, bp):
    """Host-side sharding: build the 8 per-core device input maps."""
    import ml_dtypes
    bf16 = ml_dtypes.bfloat16
    KD8 = D // P
    x = np.asarray(x, dtype=np.float32)
    scale = np.float32(1.0 / np.sqrt(HD))
    xT = x.reshape(BS, D).T.astype(bf16)  # [D, BS]
    # pre-chunked layout: [p, chunk, o, m] = xT[o*128+p, chunk*512+m]
    xTc = np.ascontiguousarray(
        xT.reshape(D // P, P, NCHUNK, 512).transpose(1, 2, 0, 3))
    mb = (np.asarray(attention_mask).astype(np.float32) - 1.0) * np.float32(1e9)
    mb = np.ascontiguousarray(
        mb.reshape(B, NB, P).transpose(2, 0, 1).reshape(P, B * NB))
    # multiplicative causal masks: 1 where 128*j + p <= q', else 0;
    # duplicated for the two heads: [128, 4, 2, 512]
    pp = np.arange(P)[:, None]
    ff = np.arange(512)[None, :]
    cm = np.stack(
        [np.where(P * j + pp <= ff, 1.0, 0.0).astype(bf16)
         for j in range(4)], axis=1)  # [128, 4, 512]
    cm = np.ascontiguousarray(
        np.broadcast_to(cm[:, :, None, :], (P, 4, HPC, 512)))

    Wq = (np.asarray(Wq, np.float32) * scale).astype(bf16)
    bq = np.asarray(bq, np.float32) * scale
    Wk = np.asarray(Wk, np.float32).astype(bf16)
    bk = np.asarray(bk, np.float32)
    Wv = np.asarray(Wv, np.float32).astype(bf16)
    bv = np.asarray(bv, np.float32)
    Wp = np.asarray(Wp, np.float32).astype(bf16)

    def wrearr(w, cs):
        # [1024, 128] core slice -> [p, o, m] = W[o*128+p, m]
        return np.ascontiguousarray(
            w[:, cs].reshape(KD8, P, P).transpose(1, 0, 2))

    in_maps = []
    for c in range(NCORES):
        cs = slice(c * P, (c + 1) * P)
        in_maps.append({
            "xT": xTc,
            "wq": wrearr(Wq, cs),
            "wk": wrearr(Wk, cs),
            "wv": wrearr(Wv, cs),
            "bq": np.ascontiguousarray(bq[cs].reshape(P, 1)),
            "bk": np.ascontiguousarray(bk[cs].reshape(P, 1)),
            "bv": np.ascontiguousarray(bv[cs].reshape(P, 1)),
            "wp": np.ascontiguousarray(Wp[cs, :]),
            "maskb": mb,
            "cmask": cm,
        })
    return in_maps


def run(inputs, trace=False, tmpdir=None):
    """Compile (cached) + run on 8 cores. Returns (output, BassKernelResults)."""
    from concourse import bass_utils
    nc = _get_nc()
    in_maps = make_in_maps(**inputs)
    kwargs = {}
    if trace:
        kwargs = dict(trace=True, tmpdir=tmpdir)
    res = bass_utils.run_bass_kernel_spmd(
        nc, in_maps, core_ids=list(range(NCORES)), **kwargs)
    acc = np.zeros((BS, D), dtype=np.float32)
    for r in res.results:
        acc += r["yp"].astype(np.float32)
    out = acc + np.asarray(inputs["bp"], np.float32)[None, :]
    return out.reshape(B, S, D), res


def kernel(**inputs) -> np.ndarray:
    out, _ = run(inputs, trace=False)
    return out


# revision 52
# speedup vs baseline: 1.0259x; 1.0083x over previous
"""MultiHeadAttention (B=4, S=2048, D=1024, H=16, causal + key mask) on 8 trn2 cores.

Sharding: Megatron-style tensor parallel over heads. Each core owns 2 heads:
column slices of Wq/Wk/Wv (D x 128), the matching row slice of Wp (128 x D).
Each core computes a partial output y_c = attn_c @ Wp_c; host sums the 8
partials and adds bp.

v10 (340986 -> 320096 ns; engine rebalance + pipeline/boundary fixes):
  - ScalarE is pure exp (strict-FIFO queue carries nothing else, so no
    head-of-line hazard): q/k/v projection evictions moved to DVE
    tensor_scalar_add (bias fused, f32 psum -> bf16 sbuf in one op); ALL
    output-projection evictions on DVE.
  - Uniform pipelined schedule: proj chunks run 1 group ahead of the
    attention that consumes them (no dead 22us batch-0 proj phase; chunks
    13-15 double as batch-3 PE filler); exp stream starts ~8us in.
  - Each group's first two score-pairs are emitted under
    tc.high_priority(offset=60) so they jump past the boundary proj/outproj
    bursts in the PE queue and the exp stream restarts promptly.
  - Diagonal blocks: only the 128-col triangular strip is mask-multiplied
    (DVE); PV is split so the wide clean columns chain exp->PV directly.
  - Normalize: f32 reciprocal_approx_fast (DVE) -> partition-broadcast via
    gpsimd DMA descriptors + muls on the otherwise-idle GpSimd (zero PE/DVE
    cost; 6-10us DMA latency hidden by the outproj drain lag). The last two
    groups instead use a short chain: bf16 row cast -> K=1 bf16 broadcast
    matmul (213ns) -> DVE muls off PSUM; the final group also skips the
    at_sb partition-shift DMA by feeding a split output projection (two
    k=64 accumulating matmuls, Wp rows 64-127 re-homed at partitions 0-63).
  - Deferred outproj drain table pushes output projections into the
    ACT-bound batch-3 windows.
  - Host-side relayouts: xT pre-chunked [128, 16, 8, 512] and weights
    [128, 8, 128] (contiguous >=2KB DMA lines; queue busy 200us -> 105us).
  - Exp ACT table preloaded with a dummy activation at t~0.
  - v3 keeps: bf16 matmuls, row-tiled concurrent score pairs, one-block
    score lookahead, PV ones-column denominator trick.

Measured bottleneck structure (per core): PE busy ~268us (true work ~210 +
LDWEIGHTS/sem exposure), ScalarE exp 152us, DVE ~196us, wall 320us. PSUM is
the hard wall (8 banks: scores 2x2 + PV accum 2 + proj/transient 2) -- it
caps score lookahead at 2 and blocks every deeper-pipelining variant tried.
"""

import numpy as np

P = 128
B, S, D, H = 4, 2048, 1024, 16
HD = D // H  # 64
NCORES = 8
HPC = H // NCORES  # 2 heads per core
BS = B * S  # 8192
NB = S // P  # 16 k-blocks per batch
NG = S // 512  # 4 q-groups per batch
NCHUNK = BS // 512  # 16 token chunks

_CACHE = {}


def _build_nc():
    import concourse.mybir as mybir
    from concourse import bacc
    from concourse.tile import TileContext
    from concourse.masks import make_identity
    from contextlib import ExitStack

    f32 = mybir.dt.float32
    bf16 = mybir.dt.bfloat16
    AF = mybir.ActivationFunctionType

    nc = bacc.Bacc("TRN2", target_bir_lowering=False, debug=False,
                   num_devices=NCORES)

    KD = D // P  # 8 contraction chunks
    # pre-chunked x^T: [p, chunk, o, m] = x^T[o*128+p, chunk*512+m]
    xT_d = nc.dram_tensor("xT", [P, NCHUNK, KD, 512], bf16,
                          kind="ExternalInput").ap()
    # weights pre-arranged [p, o, m] = W[o*128+p, m] (contiguous 2KB lines)
    wq_d = nc.dram_tensor("wq", [P, KD, P], bf16, kind="ExternalInput").ap()
    wk_d = nc.dram_tensor("wk", [P, KD, P], bf16, kind="ExternalInput").ap()
    wv_d = nc.dram_tensor("wv", [P, KD, P], bf16, kind="ExternalInput").ap()
    bq_d = nc.dram_tensor("bq", [P, 1], f32, kind="ExternalInput").ap()
    bk_d = nc.dram_tensor("bk", [P, 1], f32, kind="ExternalInput").ap()
    bv_d = nc.dram_tensor("bv", [P, 1], f32, kind="ExternalInput").ap()
    wp_d = nc.dram_tensor("wp", [P, D], bf16, kind="ExternalInput").ap()
    mb_d = nc.dram_tensor("maskb", [P, B * NB], f32, kind="ExternalInput").ap()
    cm_d = nc.dram_tensor("cmask", [P, 4, HPC, 512], bf16,
                          kind="ExternalInput").ap()
    yp_d = nc.dram_tensor("yp", [BS, D], bf16, kind="ExternalOutput").ap()

    with TileContext(nc) as tc:
        with ExitStack() as ctx:
            consts = ctx.enter_context(tc.tile_pool(name="consts", bufs=1))
            big = ctx.enter_context(tc.tile_pool(name="big", bufs=1))
            xpool = ctx.enter_context(tc.tile_pool(name="xpool", bufs=3))
            vtpool = ctx.enter_context(tc.tile_pool(name="vtpool", bufs=2))
            ptpool = ctx.enter_context(tc.tile_pool(name="ptpool", bufs=4))
            pvspool = ctx.enter_context(tc.tile_pool(name="pvs", bufs=2))
            npool = ctx.enter_context(tc.tile_pool(name="npool", bufs=3))
            ypool = ctx.enter_context(tc.tile_pool(name="ypool", bufs=6))
            psum = ctx.enter_context(
                tc.tile_pool(name="psum", bufs=2, space="PSUM"))
            sc2pool = ctx.enter_context(
                tc.tile_pool(name="sc2pool", bufs=2, space="PSUM"))
            pvpool = ctx.enter_context(
                tc.tile_pool(name="pvpool", bufs=2, space="PSUM"))

            # ---- constants (critical path first: wq/bq gate proj chunk 0)
            wq_sb = consts.tile([P, KD, P], bf16, tag="wq")
            wk_sb = consts.tile([P, KD, P], bf16, tag="wk")
            wv_sb = consts.tile([P, KD, P], bf16, tag="wv")
            bq_sb = consts.tile([P, 1], f32, tag="bq")
            bk_sb = consts.tile([P, 1], f32, tag="bk")
            bv_sb = consts.tile([P, 1], f32, tag="bv")
            nc.sync.dma_start(wq_sb[:], wq_d)
            nc.sync.dma_start(bq_sb[:], bq_d)
            nc.sync.dma_start(wk_sb[:], wk_d)
            nc.sync.dma_start(bk_sb[:], bk_d)
            nc.sync.dma_start(wv_sb[:], wv_d)
            nc.sync.dma_start(bv_sb[:], bv_d)
            wp_sb = consts.tile([P, D], bf16, tag="wp")
            nc.sync.dma_start(wp_sb[:], wp_d)
            # rows 64..127 of Wp re-homed at partitions 0..63 for the final
            # group's split output projection (lhsT/rhs base partitions must
            # match)
            wp2_sb = consts.tile([HD, D], bf16, tag="wp2")
            nc.sync.dma_start(wp2_sb[:], wp_d[HD:P, :])
            mb_sb = consts.tile([P, B * NB], f32, tag="mb")
            nc.sync.dma_start(mb_sb[:], mb_d)
            # multiplicative causal masks, [p, j, head, q'] 0/1 bf16
            cm_sb = consts.tile([P, 4, HPC, 512], bf16, tag="cm")
            nc.sync.dma_start(cm_sb[:], cm_d)
            ident = consts.tile([P, P], bf16, tag="ident")
            make_identity(nc, ident[:])
            # ones row on partition 64 (same partition as the PV denominator
            # row) -- bf16 lhsT of the reciprocal-broadcast matmul
            ones64 = consts.tile([P, HD], bf16, tag="ones64")
            nc.vector.memset(ones64[HD:HD + 1, :], 1.0)
            # scratch for the exp ACT-table preload
            warm = consts.tile([P, 1], f32, tag="warm")
            nc.scalar.activation(warm[:], bq_sb[:], AF.Exp)

            # ---- persistent activations (all bf16) ----
            qt_sb = big.tile([P, B, S], bf16, tag="qt")  # Q^T
            kt_sb = big.tile([P, B, S], bf16, tag="kt")  # K^T
            # V in [s, hd] layout + ones col: [p=s%128, b, sblock, h, 65]
            v_sb = big.tile([P, B, NB, HPC, HD + 1], bf16, tag="v")
            at_sb = big.tile([P, B, S], bf16, tag="at")  # attn^T (normalized)
            nc.vector.memset(v_sb[:, :, :, :, HD], 1.0)

            # ---- projections for one 512-row chunk of x ----
            def proj_chunk(c, split_load=False):
                b, sc = divmod(c, NG)
                xt = xpool.tile([P, KD, 512], bf16, tag="xt")
                # gpsimd DMA queue: the sync queue carries the y writes whose
                # in-queue semaphore waits would head-of-line block this load
                if split_load:
                    # head chunks: per-o sub-loads so the first proj matmul
                    # starts after ~1/8 of the transfer
                    for o in range(KD):
                        nc.gpsimd.dma_start(xt[:, o, :], xT_d[:, c, o, :])
                else:
                    nc.gpsimd.dma_start(xt[:], xT_d[:, c, :, :])
                ssl = slice(sc * 512, (sc + 1) * 512)
                for which in range(3):
                    w_sb = (wq_sb, wk_sb, wv_sb)[which]
                    ps = psum.tile([P, 512], f32, tag="ps")
                    for o in range(KD):
                        nc.tensor.matmul(
                            ps[:], lhsT=w_sb[:, o, :], rhs=xt[:, o, :],
                            start=(o == 0), stop=(o == KD - 1))
                    if which == 0:
                        nc.vector.tensor_scalar_add(qt_sb[:, b, ssl], ps[:],
                                                    bq_sb[:])
                    elif which == 1:
                        nc.vector.tensor_scalar_add(kt_sb[:, b, ssl], ps[:],
                                                    bk_sb[:])
                    else:
                        vt = vtpool.tile([P, 512], bf16, tag="vt")
                        nc.vector.tensor_scalar_add(vt[:], ps[:], bv_sb[:])
                        for t in range(4):
                            # shares the "ps" slots (pools size per tag)
                            trp = psum.tile([P, P], bf16, tag="ps")
                            nc.tensor.transpose(
                                trp[:], vt[:, t * P:(t + 1) * P], ident[:])
                            sb_i = sc * 4 + t
                            nc.vector.tensor_copy(
                                v_sb[:, b, sb_i, 0, 0:HD], trp[:, 0:HD])
                            nc.vector.tensor_copy(
                                v_sb[:, b, sb_i, 1, 0:HD],
                                trp[:, HD:2 * HD])

            # ---- output projection for one (b, g) q-group ----
            def outproj(b, g):
                for qc in range(4):
                    q0 = g * 512 + qc * P
                    r0 = b * S + q0
                    y_sb = ypool.tile([P, 2, 512], bf16, tag="y",
                                      name=f"y_{b}_{g}_{qc}")
                    for half in range(2):
                        yp_ps = psum.tile([P, 512], f32, tag="ps",
                                          name=f"yps_{b}_{g}_{qc}_{half}")
                        nc.tensor.matmul(
                            yp_ps[:],
                            lhsT=at_sb[:, b, q0:q0 + P],
                            rhs=wp_sb[:, half * 512:(half + 1) * 512],
                            start=True, stop=True)
                        # all evictions on DVE: ScalarE is strict FIFO, so
                        # an eviction waiting on its outproj matmul would
                        # head-of-line block the next group's exp
                        nc.vector.tensor_copy(y_sb[:, half, :], yp_ps[:])
                    nc.sync.dma_start(
                        yp_d[r0:r0 + P, :],
                        y_sb[:].rearrange("p a n -> p (a n)"))

            # scores for one (b, g, kb): row-tiled concurrent pair
            def scores_for(b, g, kb):
                j = kb - 4 * g
                # diagonal blocks: q < 128*j is fully masked
                qo = 128 * max(j, 0)
                sc2 = sc2pool.tile([P, HPC, 512], f32, tag="sc2",
                                   name=f"sc2_{b}_{g}_{kb}")
                for h in range(HPC):
                    hsl = slice(h * HD, (h + 1) * HD)
                    nc.tensor.matmul(
                        sc2[:, h, qo:512],
                        lhsT=kt_sb[hsl, b, kb * P:(kb + 1) * P],
                        rhs=qt_sb[hsl, b, g * 512 + qo:(g + 1) * 512],
                        start=True, stop=True)
                return sc2, kb, j, qo

            # ---- attention for one (b, g) q-group. `first` is this
            # group's pre-emitted first score-pair: the main loop emits it
            # BEFORE the previous boundary's proj/outproj bursts so the exp
            # stream restarts immediately at group boundaries ----
            def attn_group(b, g, pending, split_out=False, tail=False,
                           first=None):
                gsl = slice(g * 512, (g + 1) * 512)
                nkb = 4 * (g + 1)
                order = list(range(nkb))
                pvs = [pvpool.tile([P, 512], f32, tag="pv",
                                   name=f"pv_{b}_{g}_{h}")
                       for h in range(HPC)]

                def scores(kb):
                    return scores_for(b, g, kb)

                cur = first if first is not None else scores(order[0])
                for i, kb in enumerate(order):
                    nxt = scores(order[i + 1]) if i + 1 < nkb else None
                    sc2, _, j, qo = cur
                    col = b * NB + kb
                    pt = ptpool.tile([P, HPC, 512], bf16, tag="pt")
                    if qo == 0:
                        nc.scalar.activation(pt[:], sc2[:], AF.Exp,
                                             bias=mb_sb[:, col:col + 1])
                    else:
                        nc.scalar.activation(pt[:, :, qo:512],
                                             sc2[:, :, qo:512], AF.Exp,
                                             bias=mb_sb[:, col:col + 1])
                    if j >= 0:
                        # causal mask: only the 128-col strip [qo, qo+128)
                        # is triangular -- mask just that strip so the wide
                        # clean part of PV never waits on the mask-mul
                        nc.vector.tensor_mul(pt[:, :, qo:qo + P],
                                             pt[:, :, qo:qo + P],
                                             cm_sb[:, j, :, qo:qo + P])
                    for h in range(HPC):
                        if j >= 0 and qo + P < 512:
                            # clean columns: chain is exp -> PV directly
                            nc.tensor.matmul(
                                pvs[h][0:HD + 1, qo + P:512],
                                lhsT=v_sb[:, b, kb, h, :],
                                rhs=pt[:, h, qo + P:512],
                                start=(i == 0), stop=False)
                            nc.tensor.matmul(
                                pvs[h][0:HD + 1, qo:qo + P],
                                lhsT=v_sb[:, b, kb, h, :],
                                rhs=pt[:, h, qo:qo + P],
                                start=False, stop=(i == nkb - 1))
                        else:
                            nc.tensor.matmul(
                                pvs[h][0:HD + 1, qo:512],
                                lhsT=v_sb[:, b, kb, h, :],
                                rhs=pt[:, h, qo:512],
                                start=(i == 0), stop=(i == nkb - 1))
                    cur = nxt
                # evict PV psums immediately so the banks recycle without
                # waiting on the normalize chain
                pvs_sb = pvspool.tile([P, HPC, 512], f32, tag="pvs")
                for h in range(HPC):
                    nc.vector.tensor_copy(pvs_sb[0:HD + 1, h, :],
                                          pvs[h][0:HD + 1, :])
                if not split_out:
                    pending.append((b, g))
                # ---- normalize: 1/denom (row 64, bf16) -> K=1 bf16 matmul
                # broadcast into PSUM -> DVE muls reading PSUM ----
                # reciprocal_approx_fast misbehaves on single-partition
                # slices -- run it over the full tile (unused rows discarded)
                rcp = npool.tile([P, HPC, 512], f32, tag="rcp")
                nc.vector.reciprocal_approx_fast(rcp[:], pvs_sb[:])
                if not tail:
                    # slack-tolerant path (outproj lag hides the 6-10us DMA
                    # latency): partition-broadcast via DMA descriptors and
                    # muls on the otherwise-idle GpSimd -- zero PE cost and
                    # near-zero DVE cost
                    tmp = npool.tile([HD, 512], bf16, tag="tmp")
                    dbc = npool.tile([HD, HPC, 512], f32, tag="dbc")
                    for h in range(HPC):
                        nc.gpsimd.dma_start(
                            dbc[:, h, :],
                            rcp[HD:HD + 1, h, None, :]
                            .to_broadcast((1, HD, 512)))
                    nc.gpsimd.tensor_mul(at_sb[0:HD, b, gsl],
                                         pvs_sb[0:HD, 0, :], dbc[:, 0, :])
                    nc.gpsimd.tensor_mul(tmp[:], pvs_sb[0:HD, 1, :],
                                         dbc[:, 1, :])
                    nc.gpsimd.dma_start(at_sb[HD:2 * HD, b, gsl], tmp[:])
                    return None
                # tail groups: short matmul-broadcast chain
                # bf16 copy of the denominator-reciprocal row: keeps the
                # broadcast matmul at 1 cyc/row (fp32 rhs would be 4x)
                rcpb = npool.tile([P, HPC, 512], bf16, tag="rcpb")
                nc.vector.tensor_copy(rcpb[HD:HD + 1, :, :],
                                      rcp[HD:HD + 1, :, :])
                bc = [pvpool.tile([P, 512], f32, tag="pv",
                                  name=f"bc_{b}_{g}_{h}")
                      for h in range(HPC)]
                for h in range(HPC):
                    nc.tensor.matmul(
                        bc[h][0:HD, :], lhsT=ones64[HD:HD + 1, :],
                        rhs=rcpb[HD:HD + 1, h, :], start=True, stop=True)
                tmp = npool.tile([HD, 512], bf16, tag="tmp")
                if split_out:
                    # final group: keep both halves at partitions 0..63 and
                    # feed the split output projection directly -- no at_sb
                    # partition-shift DMA in the tail chain
                    a0 = npool.tile([HD, 512], bf16, tag="a0")
                    nc.vector.tensor_mul(a0[:], pvs_sb[0:HD, 0, :],
                                         bc[0][0:HD, :])
                    nc.vector.tensor_mul(tmp[:], pvs_sb[0:HD, 1, :],
                                         bc[1][0:HD, :])
                    return a0, tmp
                nc.vector.tensor_mul(at_sb[0:HD, b, gsl],
                                     pvs_sb[0:HD, 0, :], bc[0][0:HD, :])
                nc.vector.tensor_mul(tmp[:], pvs_sb[0:HD, 1, :],
                                     bc[1][0:HD, :])
                nc.gpsimd.dma_start(at_sb[HD:2 * HD, b, gsl], tmp[:])
                return None

            # ---- split output projection for the final group: two k=64
            # accumulating matmuls per psum, lhsT halves at partitions 0-63
            def outproj_split(b, g, a0, a1):
                for qc in range(4):
                    q0 = g * 512 + qc * P
                    r0 = b * S + q0
                    qsl = slice(qc * P, (qc + 1) * P)
                    y_sb = ypool.tile([P, 2, 512], bf16, tag="y",
                                      name=f"ys_{b}_{g}_{qc}")
                    for half in range(2):
                        nsl = slice(half * 512, (half + 1) * 512)
                        yp_ps = psum.tile([P, 512], f32, tag="ps",
                                          name=f"yss_{b}_{g}_{qc}_{half}")
                        nc.tensor.matmul(
                            yp_ps[:], lhsT=a0[:, qsl],
                            rhs=wp_sb[0:HD, nsl], start=True, stop=False)
                        nc.tensor.matmul(
                            yp_ps[:], lhsT=a1[:, qsl],
                            rhs=wp2_sb[:, nsl], start=False, stop=True)
                        nc.vector.tensor_copy(y_sb[:, half, :], yp_ps[:])
                    nc.sync.dma_start(
                        yp_d[r0:r0 + P, :],
                        y_sb[:].rearrange("p a n -> p (a n)"))

            # ---- schedule: uniform pipeline. proj chunks run 2 groups ahead
            # of the attention that consumes them, so batch-0 attention (and
            # its exp stream) starts ~8us in instead of after a dead 22us
            # proj-only phase. Output projections are deferred into the
            # ACT-bound batch-3 window via the drain table. ----
            DRAIN = [[0, 0, 1, 1], [0, 1, 1, 1], [0, 0, 1, 1], [2, 2, 2, 9]]
            proj_chunk(0, split_load=True)
            proj_chunk(1, split_load=True)
            pending = []
            first = None
            for b in range(B):
                for g in range(NG):
                    last = (b == B - 1 and g == NG - 1)
                    ret = attn_group(b, g, pending, split_out=last,
                                     tail=(b == B - 1 and g >= NG - 2),
                                     first=first)
                    # 2-ahead proj cadence: the chunk feeding group gi+1 was
                    # already emitted at gi-1, so the next group's first
                    # scores have NO dependency on this boundary's proj
                    # burst and the exp stream restarts immediately
                    c = NG * b + g + 2
                    if c < NCHUNK:
                        proj_chunk(c)
                    for _ in range(DRAIN[b][g]):
                        if pending:
                            outproj(*pending.pop(0))
                    if last:
                        outproj_split(b, g, *ret)

    nc.compile()
    return nc


def _get_nc():
    if "nc" not in _CACHE:
        _CACHE["nc"] = _build_nc()
    return _CACHE["nc"]


def make_in_maps(x, attention_mask, Wq, bq, Wk, bk, Wv, bv, Wp, bp):
    """Host-side sharding: build the 8 per-core device input maps."""
    import ml_dtypes
    bf16 = ml_dtypes.bfloat16
    KD8 = D // P
    x = np.asarray(x, dtype=np.float32)
    scale = np.float32(1.0 / np.sqrt(HD))
    xT = x.reshape(BS, D).T.astype(bf16)  # [D, BS]
    # pre-chunked layout: [p, chunk, o, m] = xT[o*128+p, chunk*512+m]
    xTc = np.ascontiguousarray(
        xT.reshape(D // P, P, NCHUNK, 512).transpose(1, 2, 0, 3))
    mb = (np.asarray(attention_mask).astype(np.float32) - 1.0) * np.float32(1e9)
    mb = np.ascontiguousarray(
        mb.reshape(B, NB, P).transpose(2, 0, 1).reshape(P, B * NB))
    # multiplicative causal masks: 1 where 128*j + p <= q', else 0;
    # duplicated for the two heads: [128, 4, 2, 512]
    pp = np.arange(P)[:, None]
    ff = np.arange(512)[None, :]
    cm = np.stack(
        [np.where(P * j + pp <= ff, 1.0, 0.0).astype(bf16)
         for j in range(4)], axis=1)  # [128, 4, 512]
    cm = np.ascontiguousarray(
        np.broadcast_to(cm[:, :, None, :], (P, 4, HPC, 512)))

    Wq = (np.asarray(Wq, np.float32) * scale).astype(bf16)
    bq = np.asarray(bq, np.float32) * scale
    Wk = np.asarray(Wk, np.float32).astype(bf16)
    bk = np.asarray(bk, np.float32)
    Wv = np.asarray(Wv, np.float32).astype(bf16)
    bv = np.asarray(bv, np.float32)
    Wp = np.asarray(Wp, np.float32).astype(bf16)

    def wrearr(w, cs):
        # [1024, 128] core slice -> [p, o, m] = W[o*128+p, m]
        return np.ascontiguousarray(
            w[:, cs].reshape(KD8, P, P).transpose(1, 0, 2))

    in_maps = []
    for c in range(NCORES):
        cs = slice(c * P, (c + 1) * P)
        in_maps.append({
            "xT": xTc,
            "wq": wrearr(Wq, cs),
            "wk": wrearr(Wk, cs),
            "wv": wrearr(Wv, cs),
            "bq": np.ascontiguousarray(bq[cs].reshape(P, 1)),
            "bk": np.ascontiguousarray(bk[cs].reshape(P, 1)),
            "bv": np.ascontiguousarray(bv[cs].reshape(P, 1)),
            "wp": np.ascontiguousarray(Wp[cs, :]),
            "maskb": mb,
            "cmask": cm,
        })
    return in_maps


def run(inputs, trace=False, tmpdir=None):
    """Compile (cached) + run on 8 cores. Returns (output, BassKernelResults)."""
    from concourse import bass_utils
    nc = _get_nc()
    in_maps = make_in_maps(**inputs)
    kwargs = {}
    if trace:
        kwargs = dict(trace=True, tmpdir=tmpdir)
    res = bass_utils.run_bass_kernel_spmd(
        nc, in_maps, core_ids=list(range(NCORES)), **kwargs)
    acc = np.zeros((BS, D), dtype=np.float32)
    for r in res.results:
        acc += r["yp"].astype(np.float32)
    out = acc + np.asarray(inputs["bp"], np.float32)[None, :]
    return out.reshape(B, S, D), res


def kernel(**inputs) -> np.ndarray:
    out, _ = run(inputs, trace=False)
    return out
